# revision 38
# baseline (speedup 1.0000x reference)
"""Trainium2 Bass kernel for nn_CR8_reg_cond_mul_5 (moe_routing).

Pipeline per pixel (B=16, C=128, H=1, W=8192; N = 131072 pixels):
  classifier: h = lrelu(bn(cl1 @ x)); x2 = lrelu(cl2 @ h); L = cl3 @ x2
  inds = argmax(L[:128]);  mask = lrelu(L[128])
  regression: r = lrelu(bn(reg1 @ x)); cat = [r; h]
  y = lrelu(cat @ w2[inds//16] + b2[inds//16])
  reg = y . w3[inds,:,0] + b3[inds];  x_real = (inds + reg) / 128

Sharding: data-parallel over batch; core c handles batches {2c, 2c+1}
(16384 pixels), weights replicated.  No collectives.

Device kernel (v2): a 7-stage software pipeline over 32 units of 512
pixels, every stage consuming only tensors produced in earlier slots so
no engine queue ever waits on same-slot work:
  S0  x DMA + xs = x * 2^-11 (for the L1 lo term)
  S1a L1 (f16 hi + subnormal-safe scaled-lo f16 term) -> h;
      R1 (f16) -> r8 (fp8); Pool casts h -> fp8 (cat k-tile 2)
  S1b L2 (f32r hi/lo x f32r h) -> x2
  S1c L3 (f32r hi/lo) -> logits l_t (f32, bias on DVE)
  S2a PE-transpose logits to pixel-major (f32, exact) -> DVE max-reduce
      -> exact-equality one-hot (bf16)
  S2b PE-transpose one-hot back to channel-major -> SBUF (f32r)
  S3  CondMul: all 8 experts via TWO fp8 DoubleRow matmuls
      (contraction 256 = [r; h] at 0.5 cyc/col); block-masked w3-table
      matmul folds expert mask + w3 gather; per-pixel dot folded into a
      third fp8 DoubleRow reduce; mask/res rows share one PSUM tile,
      one copy + one DMA per unit.

Precision engineering (relerr ~1.19e-2 vs the 2e-2 gate; x ships f16):
  - weight hi/lo splits give ~22-bit weights where argmax cares (L1 lo
    is f16 scaled by 2^11 against xs to dodge f16 subnormal flush; L2/
    L3 lo in f32r);
  - logits stay f32 through transpose/max/compare so exact-equality
    one-hot ties stay as rare as in fp32;
  - the regression branch (r, cat, w2, w3 gather, final dot) runs in
    fp8e4m3 (x16 prescale, host rescales): its output reaches x_real
    as reg/128, so fp8 noise is invisible at the gate.

Engine balance per unit (~5us slot): Scalar 6 evacuation-activations,
DVE bias/max/eq/mults/row-copy, Pool the fp8 cat cast, PE 17 matmuls
(f16/f32r at 1 cyc/col, fp8 DR at 0.5); PSUM: 2+4+2 banks for the
S1 ring, transpose/CondMul ring, and the packed output rows.

Host side (axon tunnel ~60-80 MB/s, per-array round trips): x ships as
f16, all constants pack into one [128, NCONST] f32 tensor (device-
resident across calls keyed by content hash), outputs pack into one
[BPC, UPB, 1024] f32 tensor ([mask | res] per unit); mask bias+lrelu
and the /(128*16) rescale finish on the host.
"""
import numpy as np

import concourse.bacc as bacc
import concourse.mybir as mybir
import concourse.tile as tile
from concourse.bass_utils import run_bass_kernel_spmd

F32 = mybir.dt.float32
F32R = mybir.dt.float32r
BF16 = mybir.dt.bfloat16
F16 = mybir.dt.float16
F8E4 = mybir.dt.float8e4
AF = mybir.ActivationFunctionType
ALU = mybir.AluOpType
AX = mybir.AxisListType
DR = mybir.MatmulPerfMode.DoubleRow

B, C, W = 16, 128, 8192
NCORES = 8
BPC = B // NCORES          # batches per core
U = 512                    # pixels per pipeline unit
UPB = W // U               # units per batch (16)
H = BPC * UPB              # units per core (32)
CLASSES = 128
EPS = 1e-5
W2SCALE = 16.0             # w2 prescale so fp8e4m3 sees normal range
VSCALE = 16.0              # res-row prescale (fp8 mul path); host divides

_CACHE = {}

# packed-const column layout (single [128, NCONST] f32 input)
CO_W1T = 0
CO_W2T = 128
CO_W3T = 256
CO_R1T = 384
CO_W2P = 512         # 2 groups x [128, 256] DoubleRow weight packs
CO_W3S = 1024        # 2 groups x [128, 128] block-masked w3 tables
CO_V = 1280          # iota + b3
CO_S1 = 1281
CO_B1 = 1282
CO_B2C = 1283
CO_B3C = 1284
CO_SR = 1285
CO_BR = 1286
CO_WLAST = 1287
CO_B2S = 1288        # 2 cols
NCONST = 1290


def _build_nc():
    nc = bacc.Bacc("TRN2", target_bir_lowering=False, debug=False)

    x_d = nc.dram_tensor("x", [BPC, C, W], F16, kind="ExternalInput")
    cpk_d = nc.dram_tensor("cpack", [128, NCONST], F32, kind="ExternalInput")
    # per-unit row pair [mask | res] packed side by side; host applies
    # mask bias+lrelu and the /(128*VSCALE) scale.  f32: no quantization.
    out_d = nc.dram_tensor("out", [BPC, UPB, 2 * U], F32,
                           kind="ExternalOutput")

    with tile.TileContext(nc) as tc:
        with (
            tc.tile_pool(name="consts", bufs=1) as cp,
            tc.tile_pool(name="xin", bufs=1) as xp,
            tc.tile_pool(name="work", bufs=1) as wp,
            tc.tile_pool(name="psA", bufs=2, space="PSUM") as pmA,
            tc.tile_pool(name="psB", bufs=4, space="PSUM") as pmB,
            tc.tile_pool(name="psrow", bufs=1, space="PSUM") as pr,
        ):
            # ---- setup: one DMA for every constant, on-chip casts ----
            cpk = cp.tile([128, NCONST], F32, tag="cpack")
            nc.sync.dma_start(cpk[:], cpk_d[:])

            def csl(col, n=1):
                return cpk[:, col:col + n]

            def f16split(col, name):
                # hi/lo weight split in f32r: the lo residual (~w * 2^-12)
                # underflows f16's subnormal range and would be flushed;
                # f32r's 8-bit exponent keeps it alive.  Moving operands
                # stay f16 (mixed-dtype matmul, still 1 cycle/col).
                wh = cp.tile([128, 128], F32R, tag=f"{name}_h")
                nc.vector.tensor_copy(wh[:], csl(col, 128))
                wl = cp.tile([128, 128], F32R, tag=f"{name}_l")
                nc.vector.tensor_tensor(wl[:], csl(col, 128), wh[:],
                                        ALU.subtract)
                return wh, wl

            s1 = csl(CO_S1)
            b1 = csl(CO_B1)
            b2c = csl(CO_B2C)
            b3c = csl(CO_B3C)
            sr = csl(CO_SR)
            br = csl(CO_BR)
            b2s = [csl(CO_B2S + g) for g in range(2)]

            # setup ordered by first use: w1h/r1 gate the first L1/R1
            # matmuls, so they are cast first; everything else overlaps
            # the early pipeline slots.
            w1h = cp.tile([128, 128], F16, tag="w1_h")
            nc.vector.tensor_copy(w1h[:], csl(CO_W1T, 128))
            r1_16 = cp.tile([128, 128], F16, tag="r1_16")
            nc.vector.tensor_copy(r1_16[:], csl(CO_R1T, 128))
            # L1 lo: f16((w - f16(w)) * 2^11); pairs with xs = x * 2^-11
            # (raw residual would underflow f16 subnormals)
            w1res = cp.tile([128, 128], F32, tag="w1_res")
            nc.vector.tensor_tensor(w1res[:], csl(CO_W1T, 128), w1h[:],
                                    ALU.subtract)
            w1l = cp.tile([128, 128], F16, tag="w1_l")
            nc.vector.tensor_scalar(w1l[:], w1res[:], 2048.0, None, ALU.mult)

            # identity matrices built on-device: iota(j - p) == 0
            iota_i = cp.tile([128, 128], mybir.dt.int32, tag="iota_i")
            nc.gpsimd.iota(iota_i[:], [[1, 128]], base=0,
                           channel_multiplier=-1)
            idn32t = cp.tile([128, 128], F32, tag="idn32")
            nc.gpsimd.tensor_scalar(idn32t[:], iota_i[:], 0, None,
                                    ALU.is_equal)
            idn32 = idn32t[:]

            w2h, w2l = f16split(CO_W2T, "w2")
            w3h, w3l = f16split(CO_W3T, "w3")
            idnbf = cp.tile([128, 128], BF16, tag="idnbf")
            nc.vector.tensor_copy(idnbf[:], idn32)
            wlastr = cp.tile([128, 1], F32R, tag="wlastr")
            nc.vector.tensor_copy(wlastr[:], csl(CO_WLAST))
            v_bf = cp.tile([128, 1], BF16, tag="v_bf")
            nc.vector.tensor_copy(v_bf[:], csl(CO_V))
            # DR reduce weights: [128, 2, 32] with only output row 0 = 1
            ones2_8 = cp.tile([128, 64], F8E4, tag="ones2_8")
            nc.vector.memset(ones2_8[:], 0.0)
            nc.vector.memset(ones2_8[:, 0:1], 1.0)
            nc.vector.memset(ones2_8[:, 32:33], 1.0)
            w2p8 = []
            for g in range(2):
                t = cp.tile([128, 256], F8E4, tag=f"w2p8_{g}")
                nc.vector.tensor_copy(t[:], csl(CO_W2P + g * 256, 256))
                w2p8.append(t)
            w3s = []
            for g in range(2):
                t = cp.tile([128, 128], BF16, tag=f"w3s_{g}")
                nc.vector.tensor_copy(t[:], csl(CO_W3S + g * 128, 128))
                w3s.append(t)

            st = [dict() for _ in range(H)]

            def S0(i):
                """prefetch x + scaled copy for the L1 lo term"""
                b, u = divmod(i, UPB)
                w0 = u * U
                x_t = xp.tile([128, U], F16, tag="x", bufs=5)
                nc.sync.dma_start(x_t[:], x_d[b, :, w0:w0 + U])
                xs_t = xp.tile([128, U], F16, tag="xs", bufs=5)
                if i % 2 == 0:
                    nc.vector.tensor_scalar(xs_t[:], x_t[:], 1.0 / 2048.0,
                                            None, ALU.mult)
                else:
                    nc.scalar.activation(xs_t[:], x_t[:], AF.Identity,
                                         scale=1.0 / 2048.0)
                st[i].update(x=x_t, xs=xs_t, b=b, w0=w0)

            def S1a(i):
                """L1 -> h (f32r), R1 -> r8 (fp8), h8 (Pool)"""
                s = st[i]
                ps = pmA.tile([128, U], F32, tag="mmA")
                nc.tensor.matmul(ps[:], w1h[:], s["x"][:],
                                 start=True, stop=False)
                nc.tensor.matmul(ps[:], w1l[:], s["xs"][:],
                                 start=False, stop=True)
                h_t = wp.tile([128, U], F32R, tag="h", bufs=6)
                nc.scalar.activation(h_t[:], ps[:], AF.Lrelu,
                                     bias=b1, scale=s1, alpha=0.01)

                ps = pmA.tile([128, U], F32, tag="mmA")
                nc.tensor.matmul(ps[:], r1_16[:], s["x"][:],
                                 start=True, stop=True)
                rh8_t = wp.tile([128, 2 * U], F8E4, tag="rh8", bufs=8)
                nc.scalar.activation(rh8_t[:, 0:U], ps[:], AF.Lrelu,
                                     bias=br, scale=sr, alpha=0.01)
                nc.gpsimd.tensor_copy(rh8_t[:, U:2 * U], h_t[:])
                s["h"] = h_t
                s["rh8"] = rh8_t

            def S1b(i):
                """L2 -> x2 (f32r)"""
                s = st[i]
                ps = pmA.tile([128, U], F32, tag="mmA")
                nc.tensor.matmul(ps[:], w2h[:], s["h"][:],
                                 start=True, stop=False)
                nc.tensor.matmul(ps[:], w2l[:], s["h"][:],
                                 start=False, stop=True)
                x2_t = wp.tile([128, U], F32R, tag="x2", bufs=8)
                nc.scalar.activation(x2_t[:], ps[:], AF.Lrelu,
                                     bias=b2c, alpha=0.01)
                s["x2"] = x2_t

            def S1c(i):
                """L3 -> logits l_t (f32, bias via DVE)"""
                s = st[i]
                ps = pmA.tile([128, U], F32, tag="mmA")
                nc.tensor.matmul(ps[:], w3h[:], s["x2"][:],
                                 start=True, stop=False)
                nc.tensor.matmul(ps[:], w3l[:], s["x2"][:],
                                 start=False, stop=True)
                l_t = wp.tile([128, U], F32, tag="lt", bufs=5)
                if i % 4 == 3:
                    nc.scalar.activation(l_t[:], ps[:], AF.Identity, bias=b3c)
                else:
                    nc.vector.tensor_scalar(l_t[:], ps[:], b3c, None, ALU.add)
                s["lt"] = l_t

            def S2a(i):
                """logits -> pixel-major -> per-pixel max -> one-hot"""
                s = st[i]
                ps_lt = pmB.tile([128, U], F32, tag="mmB")
                for j in range(U // 128):
                    nc.tensor.transpose(ps_lt[:, j * 128:(j + 1) * 128],
                                        s["lt"][:, j * 128:(j + 1) * 128],
                                        idn32)
                lt3 = ps_lt[:].rearrange("p (b c) -> p b c", c=128)
                nhb = U // 128
                maxv = wp.tile([128, nhb], F32, tag="maxv", bufs=3)
                nc.vector.tensor_reduce(maxv[:], lt3, AX.X, ALU.max)
                eq_t = wp.tile([128, U], BF16, tag="eq", bufs=4)
                eq3 = eq_t[:].rearrange("p (b c) -> p b c", c=128)
                maxb = maxv[:].unsqueeze(-1).broadcast_to([128, nhb, 128])
                nc.vector.tensor_tensor(eq3, lt3, maxb, ALU.is_equal)
                s["eq"] = eq_t

            def S2b(i):
                """one-hot back to channel-major via DMA-XBAR transpose:
                logical [512,128] transpose laid out [128, 4, 128] is
                exactly the blocked channel-major one-hot"""
                s = st[i]
                oh_t = wp.tile([128, U], BF16, tag="oh", bufs=4)
                oh3 = oh_t[:].rearrange("p (b c) -> p b c", c=128)
                nc.sync.dma_start(oh3, s["eq"][:], transpose=True)
                s["oh"] = oh_t

            def S3(i):
                """CondMul experts (fp8 DoubleRow), w3 select, output rows"""
                s = st[i]
                oh_t, x2_t, rh8_t = s["oh"], s["x2"], s["rh8"]
                rh3 = rh8_t[:].rearrange("p (two n) -> p two n", two=2)
                mul_t = wp.tile([128, 2 * U], F8E4, tag="mul", bufs=2)
                for g in range(2):
                    ps_ly = pmB.tile([128, U], F32, tag="mmB")
                    w3d = w2p8[g][:].rearrange("p (two m) -> p two m", two=2)
                    nc.tensor.matmul(ps_ly[:], w3d, rh3, start=True, stop=True,
                                     perf_mode=DR)
                    ly_g = wp.tile([128, U], F16, tag=f"ly{g}", bufs=2)
                    nc.scalar.activation(ly_g[:], ps_ly[:], AF.Lrelu,
                                         bias=b2s[g], scale=1.0 / W2SCALE,
                                         alpha=0.01)
                    ps_w = pmB.tile([128, U], F32, tag="mmB")
                    nc.tensor.matmul(ps_w[:], w3s[g][:], oh_t[:],
                                     start=True, stop=True)
                    # mul in fp8 x16 (w3s tables are prescaled x16);
                    # host folds the /16 into the final /128 scale
                    nc.vector.tensor_tensor(mul_t[:, g * U:(g + 1) * U],
                                            ly_g[:], ps_w[:], ALU.mult)

                b, u = s["b"], s["w0"] // U
                rows2 = pr.tile([32, 2 * U], F32, tag="rows", name="rows2")
                nc.tensor.matmul(rows2[0:1, 0:U], wlastr[:], x2_t[:],
                                 start=True, stop=True, skip_group_check=True)
                nc.tensor.matmul(rows2[0:1, U:2 * U], v_bf[:], oh_t[:],
                                 start=True, stop=False, skip_group_check=True)
                mul3 = mul_t[:].rearrange("p (two n) -> p two n", two=2)
                o3 = ones2_8[:].rearrange("p (two m) -> p two m", two=2)
                nc.tensor.matmul(rows2[0:32, U:2 * U], o3, mul3,
                                 start=False, stop=True, perf_mode=DR,
                                 skip_group_check=True)
                rw = wp.tile([1, 2 * U], F32, tag="rw", bufs=2)
                nc.vector.tensor_copy(rw[:], rows2[0:1, :])
                nc.sync.dma_start(out_d[b, u], rw[:])
                st[i] = {}

            for i in range(H + 7):
                if i < H:
                    S0(i)
                if 0 <= i - 1 < H:
                    S1a(i - 1)
                if 0 <= i - 2 < H:
                    S1b(i - 2)
                if 0 <= i - 3 < H:
                    S1c(i - 3)
                if 0 <= i - 4 < H:
                    S2a(i - 4)
                if 0 <= i - 5 < H:
                    S2b(i - 5)
                if 0 <= i - 6 < H:
                    S3(i - 6)

    nc.compile()
    return nc


def _prep_consts(inputs):
    f32 = np.float32
    cl1_w = np.asarray(inputs['cl1_w'], f32)
    cl1_b = np.asarray(inputs['cl1_b'], f32)
    g1 = np.asarray(inputs['cl1_bn_g'], f32)
    bt1 = np.asarray(inputs['cl1_bn_b'], f32)
    m1 = np.asarray(inputs['cl1_bn_m'], f32)
    v1 = np.asarray(inputs['cl1_bn_v'], f32)
    cl2_w = np.asarray(inputs['cl2_w'], f32)
    cl2_b = np.asarray(inputs['cl2_b'], f32)
    cl3_w = np.asarray(inputs['cl3_w'], f32)
    cl3_b = np.asarray(inputs['cl3_b'], f32)
    reg1_w = np.asarray(inputs['reg1_w'], f32)
    reg1_b = np.asarray(inputs['reg1_b'], f32)
    gr = np.asarray(inputs['reg1_bn_g'], f32)
    btr = np.asarray(inputs['reg1_bn_b'], f32)
    mr = np.asarray(inputs['reg1_bn_m'], f32)
    vr = np.asarray(inputs['reg1_bn_v'], f32)
    w2 = np.asarray(inputs['w2'], f32)      # [8, 256, 32]
    b2 = np.asarray(inputs['b2'], f32)      # [8, 32]
    w3 = np.asarray(inputs['w3'], f32)      # [128, 32, 1]
    b3 = np.asarray(inputs['b3'], f32)      # [128, 1]

    s1 = g1 / np.sqrt(v1 + EPS)
    b1 = (cl1_b - m1) * s1 + bt1
    srv = gr / np.sqrt(vr + EPS)
    brv = (reg1_b - mr) * srv + btr

    cpack = np.zeros((128, NCONST), f32)
    cpack[:, CO_W1T:CO_W1T + 128] = cl1_w.T
    cpack[:, CO_W2T:CO_W2T + 128] = cl2_w.T
    cpack[:, CO_W3T:CO_W3T + 128] = cl3_w[:128].T
    cpack[:, CO_R1T:CO_R1T + 128] = reg1_w.T
    for g in range(2):
        # DoubleRow pack: [:, i*128+m] = w2[4g+s, i*128+p, k], m = 32s+k
        blk = np.zeros((128, 256), f32)
        for s in range(4):
            e = 4 * g + s
            for i in range(2):
                blk[:, i * 128 + s * 32:i * 128 + (s + 1) * 32] = \
                    w2[e, i * 128:(i + 1) * 128, :]
        cpack[:, CO_W2P + g * 256:CO_W2P + (g + 1) * 256] = blk * W2SCALE
        cpack[:, CO_B2S + g][4 * 32:] = 0.0
        bcol = np.zeros(128, f32)
        for s in range(4):
            bcol[s * 32:(s + 1) * 32] = b2[4 * g + s]
        cpack[:, CO_B2S + g] = bcol
        w3sel = np.zeros((128, 128), f32)
        for s in range(4):
            c0 = g * 64 + s * 16
            w3sel[c0:c0 + 16, s * 32:(s + 1) * 32] = w3[c0:c0 + 16, :, 0]
        cpack[:, CO_W3S + g * 128:CO_W3S + (g + 1) * 128] = w3sel * VSCALE
    cpack[:, CO_V] = (np.arange(128, dtype=f32) + b3[:, 0]) * VSCALE
    cpack[:, CO_S1] = s1
    cpack[:, CO_B1] = b1
    cpack[:, CO_B2C] = cl2_b
    cpack[:, CO_B3C] = cl3_b[:128]
    cpack[:, CO_SR] = srv
    cpack[:, CO_BR] = brv
    cpack[:, CO_WLAST] = cl3_w[128]

    return {
        "cpack": cpack,
        "maskb_host": float(cl3_b[128]),
    }


_DISPATCH_CACHE = {}
_DISPATCH_MESH = {}


def _cached_dispatch(nc, n_cores):
    """run_bass_via_pjrt's axon multi-core path with the jitted program
    hoisted out and cached, so repeat kernel() calls skip the per-call
    retrace + XLA re-compile. Execution path / NEFF are identical."""
    key = (id(nc), n_cores)
    d = _DISPATCH_CACHE.get(key)
    if d is not None:
        return d
    import jax
    from jax.experimental.shard_map import shard_map
    from jax.sharding import Mesh, PartitionSpec
    from concourse import bass2jax

    bass2jax.install_neuronx_cc_hook()
    assert nc.dbg_addr is None, "debug build not supported in fast path"
    partition_name = (nc.partition_id_tensor.name
                      if nc.partition_id_tensor else None)
    in_names, out_names, out_avals, zero_specs = [], [], [], []
    for alloc in nc.m.functions[0].allocations:
        if not isinstance(alloc, mybir.MemoryLocationSet):
            continue
        name = alloc.memorylocations[0].name
        if alloc.kind == "ExternalInput":
            if name != partition_name:
                in_names.append(name)
        elif alloc.kind == "ExternalOutput":
            out_names.append(name)
            shape = tuple(alloc.tensor_shape)
            dtype = mybir.dt.np(alloc.dtype)
            out_avals.append(jax.core.ShapedArray(shape, dtype))
            zero_specs.append((shape, dtype))
    n_params = len(in_names)
    n_outs = len(out_avals)
    bind_names = list(in_names) + list(out_names)
    if partition_name is not None:
        bind_names.append(partition_name)
    donate = tuple(range(n_params, n_params + n_outs))

    def _body(*args):
        operands = list(args)
        if partition_name is not None:
            operands.append(bass2jax.partition_id_tensor())
        outs = bass2jax._bass_exec_p.bind(
            *operands,
            out_avals=tuple(out_avals),
            in_names=tuple(bind_names),
            out_names=tuple(out_names),
            lowering_input_output_aliases=(),
            sim_require_finite=True,
            sim_require_nnan=True,
            nc=nc,
        )
        return tuple(outs)

    devices = jax.devices()[:n_cores]
    assert len(devices) == n_cores
    mesh = Mesh(np.asarray(devices), ("core",))
    in_specs = (PartitionSpec("core"),) * (n_params + n_outs)
    out_specs = (PartitionSpec("core"),) * n_outs
    sharded = jax.jit(
        shard_map(_body, mesh=mesh, in_specs=in_specs,
                  out_specs=out_specs, check_rep=False),
        donate_argnums=donate, keep_unused=True,
    )
    d = (sharded, in_names, out_names, out_avals, zero_specs)
    _DISPATCH_CACHE[key] = d
    _DISPATCH_MESH[id(nc)] = mesh
    return d


_STAGE_CACHE = {}


def _stage_resident(name, per_core_arr, n_cores, mesh):
    """Content-hash keyed device residency for static (weight) arrays:
    identical bytes reuse the staged device buffer, changed bytes
    restage (replicating per core). Never applied to activations."""
    import hashlib
    import jax
    from jax.sharding import NamedSharding, PartitionSpec

    h = hashlib.sha1(per_core_arr.tobytes()).digest()
    ent = _STAGE_CACHE.get(name)
    if ent is not None and ent[0] == h:
        return ent[1]
    garr = np.tile(per_core_arr, (n_cores,) + (1,) * (per_core_arr.ndim - 1))
    buf = jax.device_put(garr, NamedSharding(mesh, PartitionSpec("core")))
    buf.block_until_ready()
    _STAGE_CACHE[name] = (h, buf)
    return buf


def _run_fast(nc, global_ins, n_cores, resident=("cpack",),
              prestaged_zeros=None):
    """global_ins: name -> global (n_cores*d0, ...) array, except
    names in `resident`, which are per-core and replicated on miss."""
    sharded, in_names, out_names, out_avals, zero_specs = _cached_dispatch(
        nc, n_cores)
    args = []
    for name in in_names:
        arr = global_ins[name]
        if name in resident:
            arr = _stage_resident(name, arr, n_cores, _DISPATCH_MESH[id(nc)])
        args.append(arr)
    concat_zeros = prestaged_zeros
    if concat_zeros is None:
        concat_zeros = [
            np.zeros((n_cores * shape[0], *shape[1:]), dtype)
            for shape, dtype in zero_specs
        ]
    out_arrs = sharded(*args, *concat_zeros)
    return {
        name: np.asarray(out_arrs[i]).reshape(n_cores, *out_avals[i].shape)
        for i, name in enumerate(out_names)
    }


def _run(inputs, trace=False, **kw):
    key = "nc_v2"
    if key not in _CACHE:
        _CACHE[key] = _build_nc()
    nc = _CACHE[key]

    consts = _prep_consts(inputs)
    maskb = consts.pop("maskb_host")
    cpack = consts["cpack"]
    x_in = np.asarray(inputs['x_in'], np.float32).reshape(B, C, W)

    if trace or kw:
        x_ship = x_in.astype(np.float16)
        in_maps = []
        for c in range(NCORES):
            m = {"cpack": cpack, "x": x_ship[c * BPC:(c + 1) * BPC]}
            in_maps.append(m)
        res = run_bass_kernel_spmd(nc, in_maps, list(range(NCORES)),
                                   trace=trace, **kw)
        out = np.stack([res.results[c]["out"] for c in range(NCORES)])
    else:
        # pipelined staging: enqueue the (donated, fresh-per-call)
        # zero output buffers first so their transfer hides under
        # the f16 encode, then encode each core's x slice and
        # enqueue its (async) transfer while encoding the next
        import jax
        from jax.sharding import NamedSharding, PartitionSpec
        _, _, _, _, zero_specs = _cached_dispatch(nc, NCORES)
        mesh = _DISPATCH_MESH[id(nc)]
        shd = NamedSharding(mesh, PartitionSpec("core"))
        devices = list(mesh.devices.flatten())
        zbufs = [
            jax.device_put(
                np.zeros((NCORES * shape[0], *shape[1:]), dtype), shd)
            for shape, dtype in zero_specs
        ]
        shards = [
            jax.device_put(
                x_in[c * BPC:(c + 1) * BPC].astype(np.float16),
                devices[c])
            for c in range(NCORES)
        ]
        x_ship = jax.make_array_from_single_device_arrays(
            (B, C, W), shd, shards)
        global_ins = {"cpack": cpack, "x": x_ship}
        outs = _run_fast(nc, global_ins, NCORES, prestaged_zeros=zbufs)
        out = outs["out"]             # [NCORES, 2, BPC, W]
        res = type("R", (), {"exec_time_ns": None, "mean_exec_time_ns": None,
                             "max_exec_time_core_id": None,
                             "results": outs})()

    # out dims [NCORES, BPC, UPB, 2U]: cols 0:U mask, U:2U res
    mask = out[..., 0:U].reshape(B, W).astype(np.float32)
    xr = out[..., U:2 * U].reshape(B, W).astype(np.float32)
    # host-side finishing
    mask = mask + maskb
    mask = np.where(mask >= 0, mask, 0.01 * mask)
    xr = xr * (1.0 / (CLASSES * VSCALE))
    out_xr = xr.reshape(B, 1, 1, W)
    out_mask = mask.reshape(B, 1, 1, W)
    return (out_xr, out_mask), res


def kernel(**inputs):
    (out_xr, out_mask), _ = _run(inputs)
    return (out_xr, out_mask)


# revision 39
# speedup vs baseline: 1.0657x; 1.0657x over previous
"""Trainium2 Bass kernel for nn_CR8_reg_cond_mul_5 (moe_routing).

Pipeline per pixel (B=16, C=128, H=1, W=8192; N = 131072 pixels):
  classifier: h = lrelu(bn(cl1 @ x)); x2 = lrelu(cl2 @ h); L = cl3 @ x2
  inds = argmax(L[:128]);  mask = lrelu(L[128])
  regression: r = lrelu(bn(reg1 @ x)); cat = [r; h]
  y = lrelu(cat @ w2[inds//16] + b2[inds//16])
  reg = y . w3[inds,:,0] + b3[inds];  x_real = (inds + reg) / 128

Sharding: data-parallel over batch; core c handles batches {2c, 2c+1}
(16384 pixels), weights replicated.  No collectives.

Device kernel (v2): a 7-stage software pipeline over 32 units of 512
pixels, every stage consuming only tensors produced in earlier slots so
no engine queue ever waits on same-slot work:
  S0  x DMA + xs = x * 2^-11 (for the L1 lo term)
  S1a L1 (f16 hi + subnormal-safe scaled-lo f16 term) -> h;
      R1 (f16) -> r8 (fp8); Pool casts h -> fp8 (cat k-tile 2)
  S1b L2 (f32r hi/lo x f32r h) -> x2
  S1c L3 (f32r hi/lo) -> logits l_t (f32, bias on DVE)
  S2a PE-transpose logits to pixel-major (f32, exact) -> DVE max-reduce
      -> exact-equality one-hot (bf16)
  S2b PE-transpose one-hot back to channel-major -> SBUF (f32r)
  S3  CondMul: all 8 experts via TWO fp8 DoubleRow matmuls
      (contraction 256 = [r; h] at 0.5 cyc/col); block-masked w3-table
      matmul folds expert mask + w3 gather; per-pixel dot folded into a
      third fp8 DoubleRow reduce; mask/res rows share one PSUM tile,
      one copy + one DMA per unit.

Precision engineering (relerr ~1.19e-2 vs the 2e-2 gate; x ships f16):
  - weight hi/lo splits give ~22-bit weights where argmax cares (L1 lo
    is f16 scaled by 2^11 against xs to dodge f16 subnormal flush; L2/
    L3 lo in f32r);
  - logits stay f32 through transpose/max/compare so exact-equality
    one-hot ties stay as rare as in fp32;
  - the regression branch (r, cat, w2, w3 gather, final dot) runs in
    fp8e4m3 (x16 prescale, host rescales): its output reaches x_real
    as reg/128, so fp8 noise is invisible at the gate.

Engine balance per unit (~5us slot): Scalar 6 evacuation-activations,
DVE bias/max/eq/mults/row-copy, Pool the fp8 cat cast, PE 17 matmuls
(f16/f32r at 1 cyc/col, fp8 DR at 0.5); PSUM: 2+4+2 banks for the
S1 ring, transpose/CondMul ring, and the packed output rows.

Host side (axon tunnel ~60-80 MB/s, per-array round trips): x ships as
f16, all constants pack into one [128, NCONST] f32 tensor (device-
resident across calls keyed by content hash), outputs pack into one
[BPC, UPB, 1024] f32 tensor ([mask | res] per unit); mask bias+lrelu
and the /(128*16) rescale finish on the host.
"""
import numpy as np

import concourse.bacc as bacc
import concourse.mybir as mybir
import concourse.tile as tile
from concourse.bass_utils import run_bass_kernel_spmd

F32 = mybir.dt.float32
F32R = mybir.dt.float32r
BF16 = mybir.dt.bfloat16
F16 = mybir.dt.float16
F8E4 = mybir.dt.float8e4
AF = mybir.ActivationFunctionType
ALU = mybir.AluOpType
AX = mybir.AxisListType
DR = mybir.MatmulPerfMode.DoubleRow

B, C, W = 16, 128, 8192
NCORES = 8
BPC = B // NCORES          # batches per core
U = 512                    # pixels per pipeline unit
UPB = W // U               # units per batch (16)
H = BPC * UPB              # units per core (32)
CLASSES = 128
EPS = 1e-5
W2SCALE = 16.0             # w2 prescale so fp8e4m3 sees normal range
VSCALE = 16.0              # res-row prescale (fp8 mul path); host divides

_CACHE = {}

# packed-const column layout (single [128, NCONST] f32 input)
CO_W1T = 0
CO_W2T = 128
CO_W3T = 256
CO_R1T = 384
CO_W2P = 512         # 2 groups x [128, 256] DoubleRow weight packs
CO_W3S = 1024        # 2 groups x [128, 128] block-masked w3 tables
CO_V = 1280          # iota + b3
CO_S1 = 1281
CO_B1 = 1282
CO_B2C = 1283
CO_B3C = 1284
CO_SR = 1285
CO_BR = 1286
CO_WLAST = 1287
CO_B2S = 1288        # 2 cols
NCONST = 1290


def _build_nc():
    nc = bacc.Bacc("TRN2", target_bir_lowering=False, debug=False)

    x_d = nc.dram_tensor("x", [BPC, C, W], F16, kind="ExternalInput")
    cpk_d = nc.dram_tensor("cpack", [128, NCONST], F32, kind="ExternalInput")
    # per-unit row pair [mask | res] packed side by side; host applies
    # mask bias+lrelu and the /(128*VSCALE) scale.  f32: no quantization.
    out_d = nc.dram_tensor("out", [BPC, UPB, 2 * U], F32,
                           kind="ExternalOutput")

    with tile.TileContext(nc) as tc:
        with (
            tc.tile_pool(name="consts", bufs=1) as cp,
            tc.tile_pool(name="xin", bufs=1) as xp,
            tc.tile_pool(name="work", bufs=1) as wp,
            tc.tile_pool(name="psA", bufs=2, space="PSUM") as pmA,
            tc.tile_pool(name="psB", bufs=4, space="PSUM") as pmB,
            tc.tile_pool(name="psrow", bufs=1, space="PSUM") as pr,
        ):
            # ---- setup: one DMA for every constant, on-chip casts ----
            cpk = cp.tile([128, NCONST], F32, tag="cpack")
            nc.sync.dma_start(cpk[:], cpk_d[:])

            def csl(col, n=1):
                return cpk[:, col:col + n]

            def f16split(col, name):
                # hi/lo weight split in f32r: the lo residual (~w * 2^-12)
                # underflows f16's subnormal range and would be flushed;
                # f32r's 8-bit exponent keeps it alive.  Moving operands
                # stay f16 (mixed-dtype matmul, still 1 cycle/col).
                wh = cp.tile([128, 128], F32R, tag=f"{name}_h")
                nc.vector.tensor_copy(wh[:], csl(col, 128))
                wl = cp.tile([128, 128], F32R, tag=f"{name}_l")
                nc.vector.tensor_tensor(wl[:], csl(col, 128), wh[:],
                                        ALU.subtract)
                return wh, wl

            s1 = csl(CO_S1)
            b1 = csl(CO_B1)
            b2c = csl(CO_B2C)
            b3c = csl(CO_B3C)
            sr = csl(CO_SR)
            br = csl(CO_BR)
            b2s = [csl(CO_B2S + g) for g in range(2)]

            # setup ordered by first use: w1h/r1 gate the first L1/R1
            # matmuls, so they are cast first; everything else overlaps
            # the early pipeline slots.
            w1h = cp.tile([128, 128], F16, tag="w1_h")
            nc.vector.tensor_copy(w1h[:], csl(CO_W1T, 128))
            r1_16 = cp.tile([128, 128], F16, tag="r1_16")
            nc.vector.tensor_copy(r1_16[:], csl(CO_R1T, 128))
            # L1 lo: f16((w - f16(w)) * 2^11); pairs with xs = x * 2^-11
            # (raw residual would underflow f16 subnormals)
            w1res = cp.tile([128, 128], F32, tag="w1_res")
            nc.vector.tensor_tensor(w1res[:], csl(CO_W1T, 128), w1h[:],
                                    ALU.subtract)
            w1l = cp.tile([128, 128], F16, tag="w1_l")
            nc.vector.tensor_scalar(w1l[:], w1res[:], 2048.0, None, ALU.mult)

            # identity matrices built on-device: iota(j - p) == 0
            iota_i = cp.tile([128, 128], mybir.dt.int32, tag="iota_i")
            nc.gpsimd.iota(iota_i[:], [[1, 128]], base=0,
                           channel_multiplier=-1)
            idn32t = cp.tile([128, 128], F32, tag="idn32")
            nc.gpsimd.tensor_scalar(idn32t[:], iota_i[:], 0, None,
                                    ALU.is_equal)
            idn32 = idn32t[:]

            w2h, w2l = f16split(CO_W2T, "w2")
            w3h, w3l = f16split(CO_W3T, "w3")
            idnbf = cp.tile([128, 128], BF16, tag="idnbf")
            nc.vector.tensor_copy(idnbf[:], idn32)
            wlastr = cp.tile([128, 1], F32R, tag="wlastr")
            nc.vector.tensor_copy(wlastr[:], csl(CO_WLAST))
            v_bf = cp.tile([128, 1], BF16, tag="v_bf")
            nc.vector.tensor_copy(v_bf[:], csl(CO_V))
            # DR reduce weights: [128, 2, 32] with only output row 0 = 1
            ones2_8 = cp.tile([128, 64], F8E4, tag="ones2_8")
            nc.vector.memset(ones2_8[:], 0.0)
            nc.vector.memset(ones2_8[:, 0:1], 1.0)
            nc.vector.memset(ones2_8[:, 32:33], 1.0)
            w2p8 = []
            for g in range(2):
                t = cp.tile([128, 256], F8E4, tag=f"w2p8_{g}")
                nc.vector.tensor_copy(t[:], csl(CO_W2P + g * 256, 256))
                w2p8.append(t)
            w3s = []
            for g in range(2):
                t = cp.tile([128, 128], BF16, tag=f"w3s_{g}")
                nc.vector.tensor_copy(t[:], csl(CO_W3S + g * 128, 128))
                w3s.append(t)

            st = [dict() for _ in range(H)]

            def S0(i):
                """prefetch x + scaled copy for the L1 lo term"""
                b, u = divmod(i, UPB)
                w0 = u * U
                x_t = xp.tile([128, U], F16, tag="x", bufs=5)
                nc.sync.dma_start(x_t[:], x_d[b, :, w0:w0 + U])
                xs_t = xp.tile([128, U], F16, tag="xs", bufs=5)
                if i % 2 == 0:
                    nc.vector.tensor_scalar(xs_t[:], x_t[:], 1.0 / 2048.0,
                                            None, ALU.mult)
                else:
                    nc.scalar.activation(xs_t[:], x_t[:], AF.Identity,
                                         scale=1.0 / 2048.0)
                st[i].update(x=x_t, xs=xs_t, b=b, w0=w0)

            def S1a(i):
                """L1 -> h (f32r), R1 -> r8 (fp8), h8 (Pool)"""
                s = st[i]
                ps = pmA.tile([128, U], F32, tag="mmA")
                nc.tensor.matmul(ps[:], w1h[:], s["x"][:],
                                 start=True, stop=False)
                nc.tensor.matmul(ps[:], w1l[:], s["xs"][:],
                                 start=False, stop=True)
                h_t = wp.tile([128, U], F32R, tag="h", bufs=6)
                nc.scalar.activation(h_t[:], ps[:], AF.Lrelu,
                                     bias=b1, scale=s1, alpha=0.01)

                ps = pmA.tile([128, U], F32, tag="mmA")
                nc.tensor.matmul(ps[:], r1_16[:], s["x"][:],
                                 start=True, stop=True)
                rh8_t = wp.tile([128, 2 * U], F8E4, tag="rh8", bufs=8)
                nc.scalar.activation(rh8_t[:, 0:U], ps[:], AF.Lrelu,
                                     bias=br, scale=sr, alpha=0.01)
                nc.gpsimd.tensor_copy(rh8_t[:, U:2 * U], h_t[:])
                s["h"] = h_t
                s["rh8"] = rh8_t

            def S1b(i):
                """L2 -> x2 (f32r)"""
                s = st[i]
                ps = pmA.tile([128, U], F32, tag="mmA")
                nc.tensor.matmul(ps[:], w2h[:], s["h"][:],
                                 start=True, stop=False)
                nc.tensor.matmul(ps[:], w2l[:], s["h"][:],
                                 start=False, stop=True)
                x2_t = wp.tile([128, U], F32R, tag="x2", bufs=8)
                nc.scalar.activation(x2_t[:], ps[:], AF.Lrelu,
                                     bias=b2c, alpha=0.01)
                s["x2"] = x2_t

            def S1c(i):
                """L3 -> logits l_t (f32, bias via DVE)"""
                s = st[i]
                ps = pmA.tile([128, U], F32, tag="mmA")
                nc.tensor.matmul(ps[:], w3h[:], s["x2"][:],
                                 start=True, stop=False)
                nc.tensor.matmul(ps[:], w3l[:], s["x2"][:],
                                 start=False, stop=True)
                l_t = wp.tile([128, U], F32, tag="lt", bufs=5)
                if i % 4 == 3:
                    nc.scalar.activation(l_t[:], ps[:], AF.Identity, bias=b3c)
                else:
                    nc.vector.tensor_scalar(l_t[:], ps[:], b3c, None, ALU.add)
                s["lt"] = l_t

            def S2a(i):
                """logits -> pixel-major -> per-pixel max -> one-hot"""
                s = st[i]
                ps_lt = pmB.tile([128, U], F32, tag="mmB")
                for j in range(U // 128):
                    nc.tensor.transpose(ps_lt[:, j * 128:(j + 1) * 128],
                                        s["lt"][:, j * 128:(j + 1) * 128],
                                        idn32)
                lt3 = ps_lt[:].rearrange("p (b c) -> p b c", c=128)
                nhb = U // 128
                maxv = wp.tile([128, nhb], F32, tag="maxv", bufs=3)
                nc.vector.tensor_reduce(maxv[:], lt3, AX.X, ALU.max)
                eq_t = wp.tile([128, U], BF16, tag="eq", bufs=4)
                eq3 = eq_t[:].rearrange("p (b c) -> p b c", c=128)
                maxb = maxv[:].unsqueeze(-1).broadcast_to([128, nhb, 128])
                nc.vector.tensor_tensor(eq3, lt3, maxb, ALU.is_equal)
                s["eq"] = eq_t

            def S2b(i):
                """one-hot back to channel-major via DMA-XBAR transpose:
                logical [512,128] transpose laid out [128, 4, 128] is
                exactly the blocked channel-major one-hot"""
                s = st[i]
                oh_t = wp.tile([128, U], BF16, tag="oh", bufs=4)
                oh3 = oh_t[:].rearrange("p (b c) -> p b c", c=128)
                nc.sync.dma_start(oh3, s["eq"][:], transpose=True)
                s["oh"] = oh_t

            def S3(i):
                """CondMul experts (fp8 DoubleRow), w3 select, output rows"""
                s = st[i]
                oh_t, x2_t, rh8_t = s["oh"], s["x2"], s["rh8"]
                rh3 = rh8_t[:].rearrange("p (two n) -> p two n", two=2)
                mul_t = wp.tile([128, 2 * U], F8E4, tag="mul", bufs=2)
                for g in range(2):
                    ps_ly = pmB.tile([128, U], F32, tag="mmB")
                    w3d = w2p8[g][:].rearrange("p (two m) -> p two m", two=2)
                    nc.tensor.matmul(ps_ly[:], w3d, rh3, start=True, stop=True,
                                     perf_mode=DR)
                    ly_g = wp.tile([128, U], F16, tag=f"ly{g}", bufs=2)
                    nc.scalar.activation(ly_g[:], ps_ly[:], AF.Lrelu,
                                         bias=b2s[g], scale=1.0 / W2SCALE,
                                         alpha=0.01)
                    ps_w = pmB.tile([128, U], F32, tag="mmB")
                    nc.tensor.matmul(ps_w[:], w3s[g][:], oh_t[:],
                                     start=True, stop=True)
                    # mul in fp8 x16 (w3s tables are prescaled x16);
                    # host folds the /16 into the final /128 scale
                    nc.vector.tensor_tensor(mul_t[:, g * U:(g + 1) * U],
                                            ly_g[:], ps_w[:], ALU.mult)

                b, u = s["b"], s["w0"] // U
                rows2 = pr.tile([32, 2 * U], F32, tag="rows", name="rows2")
                nc.tensor.matmul(rows2[0:1, 0:U], wlastr[:], x2_t[:],
                                 start=True, stop=True, skip_group_check=True)
                nc.tensor.matmul(rows2[0:1, U:2 * U], v_bf[:], oh_t[:],
                                 start=True, stop=False, skip_group_check=True)
                mul3 = mul_t[:].rearrange("p (two n) -> p two n", two=2)
                o3 = ones2_8[:].rearrange("p (two m) -> p two m", two=2)
                nc.tensor.matmul(rows2[0:32, U:2 * U], o3, mul3,
                                 start=False, stop=True, perf_mode=DR,
                                 skip_group_check=True)
                rw = wp.tile([1, 2 * U], F32, tag="rw", bufs=2)
                nc.vector.tensor_copy(rw[:], rows2[0:1, :])
                nc.sync.dma_start(out_d[b, u], rw[:])
                st[i] = {}

            for i in range(H + 8):
                if i < H:
                    S0(i)
                if 0 <= i - 1 < H:
                    S1a(i - 1)
                if 0 <= i - 2 < H:
                    S1b(i - 2)
                if 0 <= i - 3 < H:
                    S1c(i - 3)
                if 0 <= i - 4 < H:
                    S2a(i - 4)
                if 0 <= i - 5 < H:
                    S2b(i - 5)
                if 0 <= i - 7 < H:
                    S3(i - 7)

    nc.compile()
    return nc


def _prep_consts(inputs):
    f32 = np.float32
    cl1_w = np.asarray(inputs['cl1_w'], f32)
    cl1_b = np.asarray(inputs['cl1_b'], f32)
    g1 = np.asarray(inputs['cl1_bn_g'], f32)
    bt1 = np.asarray(inputs['cl1_bn_b'], f32)
    m1 = np.asarray(inputs['cl1_bn_m'], f32)
    v1 = np.asarray(inputs['cl1_bn_v'], f32)
    cl2_w = np.asarray(inputs['cl2_w'], f32)
    cl2_b = np.asarray(inputs['cl2_b'], f32)
    cl3_w = np.asarray(inputs['cl3_w'], f32)
    cl3_b = np.asarray(inputs['cl3_b'], f32)
    reg1_w = np.asarray(inputs['reg1_w'], f32)
    reg1_b = np.asarray(inputs['reg1_b'], f32)
    gr = np.asarray(inputs['reg1_bn_g'], f32)
    btr = np.asarray(inputs['reg1_bn_b'], f32)
    mr = np.asarray(inputs['reg1_bn_m'], f32)
    vr = np.asarray(inputs['reg1_bn_v'], f32)
    w2 = np.asarray(inputs['w2'], f32)      # [8, 256, 32]
    b2 = np.asarray(inputs['b2'], f32)      # [8, 32]
    w3 = np.asarray(inputs['w3'], f32)      # [128, 32, 1]
    b3 = np.asarray(inputs['b3'], f32)      # [128, 1]

    s1 = g1 / np.sqrt(v1 + EPS)
    b1 = (cl1_b - m1) * s1 + bt1
    srv = gr / np.sqrt(vr + EPS)
    brv = (reg1_b - mr) * srv + btr

    cpack = np.zeros((128, NCONST), f32)
    cpack[:, CO_W1T:CO_W1T + 128] = cl1_w.T
    cpack[:, CO_W2T:CO_W2T + 128] = cl2_w.T
    cpack[:, CO_W3T:CO_W3T + 128] = cl3_w[:128].T
    cpack[:, CO_R1T:CO_R1T + 128] = reg1_w.T
    for g in range(2):
        # DoubleRow pack: [:, i*128+m] = w2[4g+s, i*128+p, k], m = 32s+k
        blk = np.zeros((128, 256), f32)
        for s in range(4):
            e = 4 * g + s
            for i in range(2):
                blk[:, i * 128 + s * 32:i * 128 + (s + 1) * 32] = \
                    w2[e, i * 128:(i + 1) * 128, :]
        cpack[:, CO_W2P + g * 256:CO_W2P + (g + 1) * 256] = blk * W2SCALE
        cpack[:, CO_B2S + g][4 * 32:] = 0.0
        bcol = np.zeros(128, f32)
        for s in range(4):
            bcol[s * 32:(s + 1) * 32] = b2[4 * g + s]
        cpack[:, CO_B2S + g] = bcol
        w3sel = np.zeros((128, 128), f32)
        for s in range(4):
            c0 = g * 64 + s * 16
            w3sel[c0:c0 + 16, s * 32:(s + 1) * 32] = w3[c0:c0 + 16, :, 0]
        cpack[:, CO_W3S + g * 128:CO_W3S + (g + 1) * 128] = w3sel * VSCALE
    cpack[:, CO_V] = (np.arange(128, dtype=f32) + b3[:, 0]) * VSCALE
    cpack[:, CO_S1] = s1
    cpack[:, CO_B1] = b1
    cpack[:, CO_B2C] = cl2_b
    cpack[:, CO_B3C] = cl3_b[:128]
    cpack[:, CO_SR] = srv
    cpack[:, CO_BR] = brv
    cpack[:, CO_WLAST] = cl3_w[128]

    return {
        "cpack": cpack,
        "maskb_host": float(cl3_b[128]),
    }


_DISPATCH_CACHE = {}
_DISPATCH_MESH = {}


def _cached_dispatch(nc, n_cores):
    """run_bass_via_pjrt's axon multi-core path with the jitted program
    hoisted out and cached, so repeat kernel() calls skip the per-call
    retrace + XLA re-compile. Execution path / NEFF are identical."""
    key = (id(nc), n_cores)
    d = _DISPATCH_CACHE.get(key)
    if d is not None:
        return d
    import jax
    from jax.experimental.shard_map import shard_map
    from jax.sharding import Mesh, PartitionSpec
    from concourse import bass2jax

    bass2jax.install_neuronx_cc_hook()
    assert nc.dbg_addr is None, "debug build not supported in fast path"
    partition_name = (nc.partition_id_tensor.name
                      if nc.partition_id_tensor else None)
    in_names, out_names, out_avals, zero_specs = [], [], [], []
    for alloc in nc.m.functions[0].allocations:
        if not isinstance(alloc, mybir.MemoryLocationSet):
            continue
        name = alloc.memorylocations[0].name
        if alloc.kind == "ExternalInput":
            if name != partition_name:
                in_names.append(name)
        elif alloc.kind == "ExternalOutput":
            out_names.append(name)
            shape = tuple(alloc.tensor_shape)
            dtype = mybir.dt.np(alloc.dtype)
            out_avals.append(jax.core.ShapedArray(shape, dtype))
            zero_specs.append((shape, dtype))
    n_params = len(in_names)
    n_outs = len(out_avals)
    bind_names = list(in_names) + list(out_names)
    if partition_name is not None:
        bind_names.append(partition_name)
    donate = tuple(range(n_params, n_params + n_outs))

    def _body(*args):
        operands = list(args)
        if partition_name is not None:
            operands.append(bass2jax.partition_id_tensor())
        outs = bass2jax._bass_exec_p.bind(
            *operands,
            out_avals=tuple(out_avals),
            in_names=tuple(bind_names),
            out_names=tuple(out_names),
            lowering_input_output_aliases=(),
            sim_require_finite=True,
            sim_require_nnan=True,
            nc=nc,
        )
        return tuple(outs)

    devices = jax.devices()[:n_cores]
    assert len(devices) == n_cores
    mesh = Mesh(np.asarray(devices), ("core",))
    in_specs = (PartitionSpec("core"),) * (n_params + n_outs)
    out_specs = (PartitionSpec("core"),) * n_outs
    sharded = jax.jit(
        shard_map(_body, mesh=mesh, in_specs=in_specs,
                  out_specs=out_specs, check_rep=False),
        donate_argnums=donate, keep_unused=True,
    )
    d = (sharded, in_names, out_names, out_avals, zero_specs)
    _DISPATCH_CACHE[key] = d
    _DISPATCH_MESH[id(nc)] = mesh
    return d


_STAGE_CACHE = {}


def _stage_resident(name, per_core_arr, n_cores, mesh):
    """Content-hash keyed device residency for static (weight) arrays:
    identical bytes reuse the staged device buffer, changed bytes
    restage (replicating per core). Never applied to activations."""
    import hashlib
    import jax
    from jax.sharding import NamedSharding, PartitionSpec

    h = hashlib.sha1(per_core_arr.tobytes()).digest()
    ent = _STAGE_CACHE.get(name)
    if ent is not None and ent[0] == h:
        return ent[1]
    garr = np.tile(per_core_arr, (n_cores,) + (1,) * (per_core_arr.ndim - 1))
    buf = jax.device_put(garr, NamedSharding(mesh, PartitionSpec("core")))
    buf.block_until_ready()
    _STAGE_CACHE[name] = (h, buf)
    return buf


def _run_fast(nc, global_ins, n_cores, resident=("cpack",),
              prestaged_zeros=None):
    """global_ins: name -> global (n_cores*d0, ...) array, except
    names in `resident`, which are per-core and replicated on miss."""
    sharded, in_names, out_names, out_avals, zero_specs = _cached_dispatch(
        nc, n_cores)
    args = []
    for name in in_names:
        arr = global_ins[name]
        if name in resident:
            arr = _stage_resident(name, arr, n_cores, _DISPATCH_MESH[id(nc)])
        args.append(arr)
    concat_zeros = prestaged_zeros
    if concat_zeros is None:
        concat_zeros = [
            np.zeros((n_cores * shape[0], *shape[1:]), dtype)
            for shape, dtype in zero_specs
        ]
    out_arrs = sharded(*args, *concat_zeros)
    return {
        name: np.asarray(out_arrs[i]).reshape(n_cores, *out_avals[i].shape)
        for i, name in enumerate(out_names)
    }


def _run(inputs, trace=False, **kw):
    key = "nc_v2"
    if key not in _CACHE:
        _CACHE[key] = _build_nc()
    nc = _CACHE[key]

    consts = _prep_consts(inputs)
    maskb = consts.pop("maskb_host")
    cpack = consts["cpack"]
    x_in = np.asarray(inputs['x_in'], np.float32).reshape(B, C, W)

    if trace or kw:
        x_ship = x_in.astype(np.float16)
        in_maps = []
        for c in range(NCORES):
            m = {"cpack": cpack, "x": x_ship[c * BPC:(c + 1) * BPC]}
            in_maps.append(m)
        res = run_bass_kernel_spmd(nc, in_maps, list(range(NCORES)),
                                   trace=trace, **kw)
        out = np.stack([res.results[c]["out"] for c in range(NCORES)])
    else:
        # pipelined staging: enqueue the (donated, fresh-per-call)
        # zero output buffers first so their transfer hides under
        # the f16 encode, then encode each core's x slice and
        # enqueue its (async) transfer while encoding the next
        import jax
        from jax.sharding import NamedSharding, PartitionSpec
        _, _, _, _, zero_specs = _cached_dispatch(nc, NCORES)
        mesh = _DISPATCH_MESH[id(nc)]
        shd = NamedSharding(mesh, PartitionSpec("core"))
        devices = list(mesh.devices.flatten())
        zbufs = [
            jax.device_put(
                np.zeros((NCORES * shape[0], *shape[1:]), dtype), shd)
            for shape, dtype in zero_specs
        ]
        shards = [
            jax.device_put(
                x_in[c * BPC:(c + 1) * BPC].astype(np.float16),
                devices[c])
            for c in range(NCORES)
        ]
        x_ship = jax.make_array_from_single_device_arrays(
            (B, C, W), shd, shards)
        global_ins = {"cpack": cpack, "x": x_ship}
        outs = _run_fast(nc, global_ins, NCORES, prestaged_zeros=zbufs)
        out = outs["out"]             # [NCORES, 2, BPC, W]
        res = type("R", (), {"exec_time_ns": None, "mean_exec_time_ns": None,
                             "max_exec_time_core_id": None,
                             "results": outs})()

    # out dims [NCORES, BPC, UPB, 2U]: cols 0:U mask, U:2U res
    mask = out[..., 0:U].reshape(B, W).astype(np.float32)
    xr = out[..., U:2 * U].reshape(B, W).astype(np.float32)
    # host-side finishing
    mask = mask + maskb
    mask = np.where(mask >= 0, mask, 0.01 * mask)
    xr = xr * (1.0 / (CLASSES * VSCALE))
    out_xr = xr.reshape(B, 1, 1, W)
    out_mask = mask.reshape(B, 1, 1, W)
    return (out_xr, out_mask), res


def kernel(**inputs):
    (out_xr, out_mask), _ = _run(inputs)
    return (out_xr, out_mask)


# revision 40
# speedup vs baseline: 1.1750x; 1.1025x over previous
"""Trainium2 Bass kernel for nn_CR8_reg_cond_mul_5 (moe_routing).

Pipeline per pixel (B=16, C=128, H=1, W=8192; N = 131072 pixels):
  classifier: h = lrelu(bn(cl1 @ x)); x2 = lrelu(cl2 @ h); L = cl3 @ x2
  inds = argmax(L[:128]);  mask = lrelu(L[128])
  regression: r = lrelu(bn(reg1 @ x)); cat = [r; h]
  y = lrelu(cat @ w2[inds//16] + b2[inds//16])
  reg = y . w3[inds,:,0] + b3[inds];  x_real = (inds + reg) / 128

Sharding: data-parallel over batch; core c handles batches {2c, 2c+1}
(16384 pixels), weights replicated.  No collectives.

Device kernel (v2): a 7-stage software pipeline over 32 units of 512
pixels, every stage consuming only tensors produced in earlier slots so
no engine queue ever waits on same-slot work:
  S0  x DMA + xs = x * 2^-11 (for the L1 lo term)
  S1a L1 (f16 hi + subnormal-safe scaled-lo f16 term) -> h;
      R1 (f16) -> r8 (fp8); Pool casts h -> fp8 (cat k-tile 2)
  S1b L2 (f32r hi/lo x f32r h) -> x2
  S1c L3 (f32r hi/lo) -> logits l_t (f32, bias on DVE)
  S2a PE-transpose logits to pixel-major (f32, exact) -> DVE max-reduce
      -> exact-equality one-hot (bf16)
  S2b PE-transpose one-hot back to channel-major -> SBUF (f32r)
  S3  CondMul: all 8 experts via TWO fp8 DoubleRow matmuls
      (contraction 256 = [r; h] at 0.5 cyc/col); block-masked w3-table
      matmul folds expert mask + w3 gather; per-pixel dot folded into a
      third fp8 DoubleRow reduce; mask/res rows share one PSUM tile,
      one copy + one DMA per unit.

Precision engineering (relerr ~1.19e-2 vs the 2e-2 gate; x ships f16):
  - weight hi/lo splits give ~22-bit weights where argmax cares (L1 lo
    is f16 scaled by 2^11 against xs to dodge f16 subnormal flush; L2/
    L3 lo in f32r);
  - logits stay f32 through transpose/max/compare so exact-equality
    one-hot ties stay as rare as in fp32;
  - the regression branch (r, cat, w2, w3 gather, final dot) runs in
    fp8e4m3 (x16 prescale, host rescales): its output reaches x_real
    as reg/128, so fp8 noise is invisible at the gate.

Engine balance per unit (~5us slot): Scalar 6 evacuation-activations,
DVE bias/max/eq/mults/row-copy, Pool the fp8 cat cast, PE 17 matmuls
(f16/f32r at 1 cyc/col, fp8 DR at 0.5); PSUM: 2+4+2 banks for the
S1 ring, transpose/CondMul ring, and the packed output rows.

Host side (axon tunnel ~60-80 MB/s, per-array round trips): x ships as
f16, all constants pack into one [128, NCONST] f32 tensor (device-
resident across calls keyed by content hash), outputs pack into one
[BPC, UPB, 1024] f32 tensor ([mask | res] per unit); mask bias+lrelu
and the /(128*16) rescale finish on the host.
"""
import numpy as np

import concourse.bacc as bacc
import concourse.mybir as mybir
import concourse.tile as tile
from concourse.bass_utils import run_bass_kernel_spmd

F32 = mybir.dt.float32
F32R = mybir.dt.float32r
BF16 = mybir.dt.bfloat16
F16 = mybir.dt.float16
F8E4 = mybir.dt.float8e4
AF = mybir.ActivationFunctionType
ALU = mybir.AluOpType
AX = mybir.AxisListType
DR = mybir.MatmulPerfMode.DoubleRow

B, C, W = 16, 128, 8192
NCORES = 8
BPC = B // NCORES          # batches per core
U = 512                    # pixels per pipeline unit
UPB = W // U               # units per batch (16)
H = BPC * UPB              # units per core (32)
CLASSES = 128
EPS = 1e-5
W2SCALE = 16.0             # w2 prescale so fp8e4m3 sees normal range
VSCALE = 16.0              # res-row prescale (fp8 mul path); host divides

_CACHE = {}

# packed-const column layout (single [128, NCONST] f32 input)
CO_W1T = 0
CO_W2T = 128
CO_W3T = 256
CO_R1T = 384
CO_W2P = 512         # 2 groups x [128, 256] DoubleRow weight packs
CO_W3S = 1024        # 2 groups x [128, 128] block-masked w3 tables
CO_V = 1280          # iota + b3
CO_S1 = 1281
CO_B1 = 1282
CO_B2C = 1283
CO_B3C = 1284
CO_SR = 1285
CO_BR = 1286
CO_WLAST = 1287
CO_B2S = 1288        # 2 cols
NCONST = 1290


def _build_nc():
    nc = bacc.Bacc("TRN2", target_bir_lowering=False, debug=False)

    x_d = nc.dram_tensor("x", [BPC, C, W], F16, kind="ExternalInput")
    cpk_d = nc.dram_tensor("cpack", [128, NCONST], F32, kind="ExternalInput")
    # per-unit row pair [mask | res] packed side by side; host applies
    # mask bias+lrelu and the /(128*VSCALE) scale.  f32: no quantization.
    out_d = nc.dram_tensor("out", [BPC, UPB, 2 * U], F32,
                           kind="ExternalOutput")

    with tile.TileContext(nc) as tc:
        with (
            tc.tile_pool(name="consts", bufs=1) as cp,
            tc.tile_pool(name="xin", bufs=1) as xp,
            tc.tile_pool(name="work", bufs=1) as wp,
            tc.tile_pool(name="psA", bufs=2, space="PSUM") as pmA,
            tc.tile_pool(name="psB", bufs=4, space="PSUM") as pmB,
            tc.tile_pool(name="psrow", bufs=1, space="PSUM") as pr,
        ):
            # ---- setup: one DMA for every constant, on-chip casts ----
            cpk = cp.tile([128, NCONST], F32, tag="cpack")
            nc.sync.dma_start(cpk[:], cpk_d[:])

            def csl(col, n=1):
                return cpk[:, col:col + n]

            def f16split(col, name):
                # hi/lo weight split in f32r: the lo residual (~w * 2^-12)
                # underflows f16's subnormal range and would be flushed;
                # f32r's 8-bit exponent keeps it alive.  Moving operands
                # stay f16 (mixed-dtype matmul, still 1 cycle/col).
                wh = cp.tile([128, 128], F32R, tag=f"{name}_h")
                nc.vector.tensor_copy(wh[:], csl(col, 128))
                wl = cp.tile([128, 128], F32R, tag=f"{name}_l")
                nc.vector.tensor_tensor(wl[:], csl(col, 128), wh[:],
                                        ALU.subtract)
                return wh, wl

            s1 = csl(CO_S1)
            b1 = csl(CO_B1)
            b2c = csl(CO_B2C)
            b3c = csl(CO_B3C)
            sr = csl(CO_SR)
            br = csl(CO_BR)
            b2s = [csl(CO_B2S + g) for g in range(2)]

            # setup ordered by first use: w1h/r1 gate the first L1/R1
            # matmuls, so they are cast first; everything else overlaps
            # the early pipeline slots.
            w1h = cp.tile([128, 128], F16, tag="w1_h")
            nc.vector.tensor_copy(w1h[:], csl(CO_W1T, 128))
            r1_16 = cp.tile([128, 128], F16, tag="r1_16")
            nc.vector.tensor_copy(r1_16[:], csl(CO_R1T, 128))
            # L1 lo: f16((w - f16(w)) * 2^11); pairs with xs = x * 2^-11
            # (raw residual would underflow f16 subnormals)
            w1res = cp.tile([128, 128], F32, tag="w1_res")
            nc.vector.tensor_tensor(w1res[:], csl(CO_W1T, 128), w1h[:],
                                    ALU.subtract)
            w1l = cp.tile([128, 128], F16, tag="w1_l")
            nc.vector.tensor_scalar(w1l[:], w1res[:], 2048.0, None, ALU.mult)

            # identity matrices built on-device: iota(j - p) == 0
            iota_i = cp.tile([128, 128], mybir.dt.int32, tag="iota_i")
            nc.gpsimd.iota(iota_i[:], [[1, 128]], base=0,
                           channel_multiplier=-1)
            idn32t = cp.tile([128, 128], F32, tag="idn32")
            nc.gpsimd.tensor_scalar(idn32t[:], iota_i[:], 0, None,
                                    ALU.is_equal)
            idn32 = idn32t[:]

            w2h, w2l = f16split(CO_W2T, "w2")
            w3h, w3l = f16split(CO_W3T, "w3")
            idnbf = cp.tile([128, 128], BF16, tag="idnbf")
            nc.vector.tensor_copy(idnbf[:], idn32)
            wlastr = cp.tile([128, 1], F32R, tag="wlastr")
            nc.vector.tensor_copy(wlastr[:], csl(CO_WLAST))
            v32r = cp.tile([128, 1], F32R, tag="v32r")
            nc.vector.tensor_copy(v32r[:], csl(CO_V))
            # DR reduce weights: [128, 2, 32] with only output row 0 = 1
            ones2_8 = cp.tile([128, 64], F8E4, tag="ones2_8")
            nc.vector.memset(ones2_8[:], 0.0)
            nc.vector.memset(ones2_8[:, 0:1], 1.0)
            nc.vector.memset(ones2_8[:, 32:33], 1.0)
            w2p8 = []
            for g in range(2):
                t = cp.tile([128, 256], F8E4, tag=f"w2p8_{g}")
                nc.vector.tensor_copy(t[:], csl(CO_W2P + g * 256, 256))
                w2p8.append(t)
            w3s = []
            for g in range(2):
                t = cp.tile([128, 128], F32R, tag=f"w3s_{g}")
                nc.vector.tensor_copy(t[:], csl(CO_W3S + g * 128, 128))
                w3s.append(t)

            st = [dict() for _ in range(H)]

            def S0(i):
                """prefetch x + scaled copy for the L1 lo term"""
                b, u = divmod(i, UPB)
                w0 = u * U
                x_t = xp.tile([128, U], F16, tag="x", bufs=5)
                nc.sync.dma_start(x_t[:], x_d[b, :, w0:w0 + U])
                xs_t = xp.tile([128, U], F16, tag="xs", bufs=5)
                if i % 2 == 0:
                    nc.vector.tensor_scalar(xs_t[:], x_t[:], 1.0 / 2048.0,
                                            None, ALU.mult)
                else:
                    nc.scalar.activation(xs_t[:], x_t[:], AF.Identity,
                                         scale=1.0 / 2048.0)
                st[i].update(x=x_t, xs=xs_t, b=b, w0=w0)

            def S1a(i):
                """L1 -> h (f32r), R1 -> r8 (fp8), h8 (Pool)"""
                s = st[i]
                ps = pmA.tile([128, U], F32, tag="mmA")
                nc.tensor.matmul(ps[:], w1h[:], s["x"][:],
                                 start=True, stop=False)
                nc.tensor.matmul(ps[:], w1l[:], s["xs"][:],
                                 start=False, stop=True)
                h_t = wp.tile([128, U], F32R, tag="h", bufs=6)
                nc.scalar.activation(h_t[:], ps[:], AF.Lrelu,
                                     bias=b1, scale=s1, alpha=0.01)

                ps = pmA.tile([128, U], F32, tag="mmA")
                nc.tensor.matmul(ps[:], r1_16[:], s["x"][:],
                                 start=True, stop=True)
                rh8_t = wp.tile([128, 2 * U], F8E4, tag="rh8", bufs=8)
                nc.scalar.activation(rh8_t[:, 0:U], ps[:], AF.Lrelu,
                                     bias=br, scale=sr, alpha=0.01)
                nc.gpsimd.tensor_copy(rh8_t[:, U:2 * U], h_t[:])
                s["h"] = h_t
                s["rh8"] = rh8_t

            def S1b(i):
                """L2 -> x2 (f32r)"""
                s = st[i]
                ps = pmA.tile([128, U], F32, tag="mmA")
                nc.tensor.matmul(ps[:], w2h[:], s["h"][:],
                                 start=True, stop=False)
                nc.tensor.matmul(ps[:], w2l[:], s["h"][:],
                                 start=False, stop=True)
                x2_t = wp.tile([128, U], F32R, tag="x2", bufs=8)
                nc.scalar.activation(x2_t[:], ps[:], AF.Lrelu,
                                     bias=b2c, alpha=0.01)
                s["x2"] = x2_t

            def S1c(i):
                """L3 -> logits l_t (f32, bias via DVE)"""
                s = st[i]
                ps = pmA.tile([128, U], F32, tag="mmA")
                nc.tensor.matmul(ps[:], w3h[:], s["x2"][:],
                                 start=True, stop=False)
                nc.tensor.matmul(ps[:], w3l[:], s["x2"][:],
                                 start=False, stop=True)
                l_t = wp.tile([128, U], F32, tag="lt", bufs=5)
                if i % 4 == 3:
                    nc.scalar.activation(l_t[:], ps[:], AF.Identity, bias=b3c)
                else:
                    nc.vector.tensor_scalar(l_t[:], ps[:], b3c, None, ALU.add)
                s["lt"] = l_t

            def S2a(i):
                """logits -> pixel-major -> per-pixel max -> one-hot"""
                s = st[i]
                ps_lt = pmB.tile([128, U], F32, tag="mmB")
                for j in range(U // 128):
                    nc.tensor.transpose(ps_lt[:, j * 128:(j + 1) * 128],
                                        s["lt"][:, j * 128:(j + 1) * 128],
                                        idn32)
                lt3 = ps_lt[:].rearrange("p (b c) -> p b c", c=128)
                nhb = U // 128
                maxv = wp.tile([128, nhb], F32, tag="maxv", bufs=3)
                nc.vector.tensor_reduce(maxv[:], lt3, AX.X, ALU.max)
                eq_t = wp.tile([128, U], BF16, tag="eq", bufs=4)
                eq3 = eq_t[:].rearrange("p (b c) -> p b c", c=128)
                maxb = maxv[:].unsqueeze(-1).broadcast_to([128, nhb, 128])
                nc.vector.tensor_tensor(eq3, lt3, maxb, ALU.is_equal)
                s["eq"] = eq_t

            def S2b(i):
                """one-hot back to channel-major"""
                s = st[i]
                ps_oh = pmB.tile([128, U], BF16, tag="mmB")
                for j in range(U // 128):
                    nc.tensor.transpose(ps_oh[:, j * 128:(j + 1) * 128],
                                        s["eq"][:, j * 128:(j + 1) * 128],
                                        idnbf[:])
                oh_t = wp.tile([128, U], F32R, tag="oh", bufs=4)
                nc.scalar.copy(oh_t[:], ps_oh[:])
                s["oh"] = oh_t

            def S3(i):
                """CondMul experts (fp8 DoubleRow), w3 select, output rows"""
                s = st[i]
                oh_t, x2_t, rh8_t = s["oh"], s["x2"], s["rh8"]
                rh3 = rh8_t[:].rearrange("p (two n) -> p two n", two=2)
                mul_t = wp.tile([128, 2 * U], F8E4, tag="mul", bufs=2)
                for g in range(2):
                    ps_ly = pmB.tile([128, U], F32, tag="mmB")
                    w3d = w2p8[g][:].rearrange("p (two m) -> p two m", two=2)
                    nc.tensor.matmul(ps_ly[:], w3d, rh3, start=True, stop=True,
                                     perf_mode=DR)
                    ly_g = wp.tile([128, U], F16, tag=f"ly{g}", bufs=2)
                    nc.scalar.activation(ly_g[:], ps_ly[:], AF.Lrelu,
                                         bias=b2s[g], scale=1.0 / W2SCALE,
                                         alpha=0.01)
                    ps_w = pmB.tile([128, U], F32, tag="mmB")
                    nc.tensor.matmul(ps_w[:], w3s[g][:], oh_t[:],
                                     start=True, stop=True)
                    # mul in fp8 x16 (w3s tables are prescaled x16);
                    # host folds the /16 into the final /128 scale
                    nc.vector.tensor_tensor(mul_t[:, g * U:(g + 1) * U],
                                            ly_g[:], ps_w[:], ALU.mult)

                b, u = s["b"], s["w0"] // U
                rows2 = pr.tile([32, 2 * U], F32, tag="rows", name="rows2")
                nc.tensor.matmul(rows2[0:1, 0:U], wlastr[:], x2_t[:],
                                 start=True, stop=True, skip_group_check=True)
                nc.tensor.matmul(rows2[0:1, U:2 * U], v32r[:], oh_t[:],
                                 start=True, stop=False, skip_group_check=True)
                mul3 = mul_t[:].rearrange("p (two n) -> p two n", two=2)
                o3 = ones2_8[:].rearrange("p (two m) -> p two m", two=2)
                nc.tensor.matmul(rows2[0:32, U:2 * U], o3, mul3,
                                 start=False, stop=True, perf_mode=DR,
                                 skip_group_check=True)
                rw = wp.tile([1, 2 * U], F32, tag="rw", bufs=2)
                nc.vector.tensor_copy(rw[:], rows2[0:1, :])
                nc.sync.dma_start(out_d[b, u], rw[:])
                st[i] = {}

            for i in range(H + 7):
                if i < H:
                    S0(i)
                if 0 <= i - 1 < H:
                    S1a(i - 1)
                if 0 <= i - 2 < H:
                    S1b(i - 2)
                if 0 <= i - 3 < H:
                    S1c(i - 3)
                if 0 <= i - 4 < H:
                    S2a(i - 4)
                if 0 <= i - 5 < H:
                    S2b(i - 5)
                if 0 <= i - 6 < H:
                    S3(i - 6)

    nc.compile()
    return nc


def _prep_consts(inputs):
    f32 = np.float32
    cl1_w = np.asarray(inputs['cl1_w'], f32)
    cl1_b = np.asarray(inputs['cl1_b'], f32)
    g1 = np.asarray(inputs['cl1_bn_g'], f32)
    bt1 = np.asarray(inputs['cl1_bn_b'], f32)
    m1 = np.asarray(inputs['cl1_bn_m'], f32)
    v1 = np.asarray(inputs['cl1_bn_v'], f32)
    cl2_w = np.asarray(inputs['cl2_w'], f32)
    cl2_b = np.asarray(inputs['cl2_b'], f32)
    cl3_w = np.asarray(inputs['cl3_w'], f32)
    cl3_b = np.asarray(inputs['cl3_b'], f32)
    reg1_w = np.asarray(inputs['reg1_w'], f32)
    reg1_b = np.asarray(inputs['reg1_b'], f32)
    gr = np.asarray(inputs['reg1_bn_g'], f32)
    btr = np.asarray(inputs['reg1_bn_b'], f32)
    mr = np.asarray(inputs['reg1_bn_m'], f32)
    vr = np.asarray(inputs['reg1_bn_v'], f32)
    w2 = np.asarray(inputs['w2'], f32)      # [8, 256, 32]
    b2 = np.asarray(inputs['b2'], f32)      # [8, 32]
    w3 = np.asarray(inputs['w3'], f32)      # [128, 32, 1]
    b3 = np.asarray(inputs['b3'], f32)      # [128, 1]

    s1 = g1 / np.sqrt(v1 + EPS)
    b1 = (cl1_b - m1) * s1 + bt1
    srv = gr / np.sqrt(vr + EPS)
    brv = (reg1_b - mr) * srv + btr

    cpack = np.zeros((128, NCONST), f32)
    cpack[:, CO_W1T:CO_W1T + 128] = cl1_w.T
    cpack[:, CO_W2T:CO_W2T + 128] = cl2_w.T
    cpack[:, CO_W3T:CO_W3T + 128] = cl3_w[:128].T
    cpack[:, CO_R1T:CO_R1T + 128] = reg1_w.T
    for g in range(2):
        # DoubleRow pack: [:, i*128+m] = w2[4g+s, i*128+p, k], m = 32s+k
        blk = np.zeros((128, 256), f32)
        for s in range(4):
            e = 4 * g + s
            for i in range(2):
                blk[:, i * 128 + s * 32:i * 128 + (s + 1) * 32] = \
                    w2[e, i * 128:(i + 1) * 128, :]
        cpack[:, CO_W2P + g * 256:CO_W2P + (g + 1) * 256] = blk * W2SCALE
        cpack[:, CO_B2S + g][4 * 32:] = 0.0
        bcol = np.zeros(128, f32)
        for s in range(4):
            bcol[s * 32:(s + 1) * 32] = b2[4 * g + s]
        cpack[:, CO_B2S + g] = bcol
        w3sel = np.zeros((128, 128), f32)
        for s in range(4):
            c0 = g * 64 + s * 16
            w3sel[c0:c0 + 16, s * 32:(s + 1) * 32] = w3[c0:c0 + 16, :, 0]
        cpack[:, CO_W3S + g * 128:CO_W3S + (g + 1) * 128] = w3sel * VSCALE
    cpack[:, CO_V] = (np.arange(128, dtype=f32) + b3[:, 0]) * VSCALE
    cpack[:, CO_S1] = s1
    cpack[:, CO_B1] = b1
    cpack[:, CO_B2C] = cl2_b
    cpack[:, CO_B3C] = cl3_b[:128]
    cpack[:, CO_SR] = srv
    cpack[:, CO_BR] = brv
    cpack[:, CO_WLAST] = cl3_w[128]

    return {
        "cpack": cpack,
        "maskb_host": float(cl3_b[128]),
    }


_DISPATCH_CACHE = {}
_DISPATCH_MESH = {}


def _cached_dispatch(nc, n_cores):
    """run_bass_via_pjrt's axon multi-core path with the jitted program
    hoisted out and cached, so repeat kernel() calls skip the per-call
    retrace + XLA re-compile. Execution path / NEFF are identical."""
    key = (id(nc), n_cores)
    d = _DISPATCH_CACHE.get(key)
    if d is not None:
        return d
    import jax
    from jax.experimental.shard_map import shard_map
    from jax.sharding import Mesh, PartitionSpec
    from concourse import bass2jax

    bass2jax.install_neuronx_cc_hook()
    assert nc.dbg_addr is None, "debug build not supported in fast path"
    partition_name = (nc.partition_id_tensor.name
                      if nc.partition_id_tensor else None)
    in_names, out_names, out_avals, zero_specs = [], [], [], []
    for alloc in nc.m.functions[0].allocations:
        if not isinstance(alloc, mybir.MemoryLocationSet):
            continue
        name = alloc.memorylocations[0].name
        if alloc.kind == "ExternalInput":
            if name != partition_name:
                in_names.append(name)
        elif alloc.kind == "ExternalOutput":
            out_names.append(name)
            shape = tuple(alloc.tensor_shape)
            dtype = mybir.dt.np(alloc.dtype)
            out_avals.append(jax.core.ShapedArray(shape, dtype))
            zero_specs.append((shape, dtype))
    n_params = len(in_names)
    n_outs = len(out_avals)
    bind_names = list(in_names) + list(out_names)
    if partition_name is not None:
        bind_names.append(partition_name)
    donate = tuple(range(n_params, n_params + n_outs))

    def _body(*args):
        operands = list(args)
        if partition_name is not None:
            operands.append(bass2jax.partition_id_tensor())
        outs = bass2jax._bass_exec_p.bind(
            *operands,
            out_avals=tuple(out_avals),
            in_names=tuple(bind_names),
            out_names=tuple(out_names),
            lowering_input_output_aliases=(),
            sim_require_finite=True,
            sim_require_nnan=True,
            nc=nc,
        )
        return tuple(outs)

    devices = jax.devices()[:n_cores]
    assert len(devices) == n_cores
    mesh = Mesh(np.asarray(devices), ("core",))
    in_specs = (PartitionSpec("core"),) * (n_params + n_outs)
    out_specs = (PartitionSpec("core"),) * n_outs
    sharded = jax.jit(
        shard_map(_body, mesh=mesh, in_specs=in_specs,
                  out_specs=out_specs, check_rep=False),
        donate_argnums=donate, keep_unused=True,
    )
    d = (sharded, in_names, out_names, out_avals, zero_specs)
    _DISPATCH_CACHE[key] = d
    _DISPATCH_MESH[id(nc)] = mesh
    return d


_STAGE_CACHE = {}


def _stage_resident(name, per_core_arr, n_cores, mesh):
    """Content-hash keyed device residency for static (weight) arrays:
    identical bytes reuse the staged device buffer, changed bytes
    restage (replicating per core). Never applied to activations."""
    import hashlib
    import jax
    from jax.sharding import NamedSharding, PartitionSpec

    h = hashlib.sha1(per_core_arr.tobytes()).digest()
    ent = _STAGE_CACHE.get(name)
    if ent is not None and ent[0] == h:
        return ent[1]
    garr = np.tile(per_core_arr, (n_cores,) + (1,) * (per_core_arr.ndim - 1))
    buf = jax.device_put(garr, NamedSharding(mesh, PartitionSpec("core")))
    buf.block_until_ready()
    _STAGE_CACHE[name] = (h, buf)
    return buf


def _run_fast(nc, global_ins, n_cores, resident=("cpack",),
              prestaged_zeros=None):
    """global_ins: name -> global (n_cores*d0, ...) array, except
    names in `resident`, which are per-core and replicated on miss."""
    sharded, in_names, out_names, out_avals, zero_specs = _cached_dispatch(
        nc, n_cores)
    args = []
    for name in in_names:
        arr = global_ins[name]
        if name in resident:
            arr = _stage_resident(name, arr, n_cores, _DISPATCH_MESH[id(nc)])
        args.append(arr)
    concat_zeros = prestaged_zeros
    if concat_zeros is None:
        concat_zeros = [
            np.zeros((n_cores * shape[0], *shape[1:]), dtype)
            for shape, dtype in zero_specs
        ]
    out_arrs = sharded(*args, *concat_zeros)
    return {
        name: np.asarray(out_arrs[i]).reshape(n_cores, *out_avals[i].shape)
        for i, name in enumerate(out_names)
    }


def _run(inputs, trace=False, **kw):
    key = "nc_v2"
    if key not in _CACHE:
        _CACHE[key] = _build_nc()
    nc = _CACHE[key]

    consts = _prep_consts(inputs)
    maskb = consts.pop("maskb_host")
    cpack = consts["cpack"]
    x_in = np.asarray(inputs['x_in'], np.float32).reshape(B, C, W)

    if trace or kw:
        x_ship = x_in.astype(np.float16)
        in_maps = []
        for c in range(NCORES):
            m = {"cpack": cpack, "x": x_ship[c * BPC:(c + 1) * BPC]}
            in_maps.append(m)
        res = run_bass_kernel_spmd(nc, in_maps, list(range(NCORES)),
                                   trace=trace, **kw)
        out = np.stack([res.results[c]["out"] for c in range(NCORES)])
    else:
        # pipelined staging: enqueue the (donated, fresh-per-call)
        # zero output buffers first so their transfer hides under
        # the f16 encode, then encode each core's x slice and
        # enqueue its (async) transfer while encoding the next
        import jax
        from jax.sharding import NamedSharding, PartitionSpec
        _, _, _, _, zero_specs = _cached_dispatch(nc, NCORES)
        mesh = _DISPATCH_MESH[id(nc)]
        shd = NamedSharding(mesh, PartitionSpec("core"))
        devices = list(mesh.devices.flatten())
        zbufs = [
            jax.device_put(
                np.zeros((NCORES * shape[0], *shape[1:]), dtype), shd)
            for shape, dtype in zero_specs
        ]
        shards = [
            jax.device_put(
                x_in[c * BPC:(c + 1) * BPC].astype(np.float16),
                devices[c])
            for c in range(NCORES)
        ]
        x_ship = jax.make_array_from_single_device_arrays(
            (B, C, W), shd, shards)
        global_ins = {"cpack": cpack, "x": x_ship}
        outs = _run_fast(nc, global_ins, NCORES, prestaged_zeros=zbufs)
        out = outs["out"]             # [NCORES, 2, BPC, W]
        res = type("R", (), {"exec_time_ns": None, "mean_exec_time_ns": None,
                             "max_exec_time_core_id": None,
                             "results": outs})()

    # out dims [NCORES, BPC, UPB, 2U]: cols 0:U mask, U:2U res
    mask = out[..., 0:U].reshape(B, W).astype(np.float32)
    xr = out[..., U:2 * U].reshape(B, W).astype(np.float32)
    # host-side finishing
    mask = mask + maskb
    mask = np.where(mask >= 0, mask, 0.01 * mask)
    xr = xr * (1.0 / (CLASSES * VSCALE))
    out_xr = xr.reshape(B, 1, 1, W)
    out_mask = mask.reshape(B, 1, 1, W)
    return (out_xr, out_mask), res


def kernel(**inputs):
    (out_xr, out_mask), _ = _run(inputs)
    return (out_xr, out_mask)


# revision 41
# speedup vs baseline: 1.2419x; 1.0570x over previous
"""Trainium2 Bass kernel for nn_CR8_reg_cond_mul_5 (moe_routing).

Pipeline per pixel (B=16, C=128, H=1, W=8192; N = 131072 pixels):
  classifier: h = lrelu(bn(cl1 @ x)); x2 = lrelu(cl2 @ h); L = cl3 @ x2
  inds = argmax(L[:128]);  mask = lrelu(L[128])
  regression: r = lrelu(bn(reg1 @ x)); cat = [r; h]
  y = lrelu(cat @ w2[inds//16] + b2[inds//16])
  reg = y . w3[inds,:,0] + b3[inds];  x_real = (inds + reg) / 128

Sharding: data-parallel over batch; core c handles batches {2c, 2c+1}
(16384 pixels), weights replicated.  No collectives.

Device kernel (v2): a 7-stage software pipeline over 32 units of 512
pixels, every stage consuming only tensors produced in earlier slots so
no engine queue ever waits on same-slot work:
  S0  x DMA + xs = x * 2^-11 (for the L1 lo term)
  S1a L1 (f16 hi + subnormal-safe scaled-lo f16 term) -> h;
      R1 (f16) -> r8 (fp8); Pool casts h -> fp8 (cat k-tile 2)
  S1b L2 (f32r hi/lo x f32r h) -> x2
  S1c L3 (f32r hi/lo) -> logits l_t (f32, bias on DVE)
  S2a PE-transpose logits to pixel-major (f32, exact) -> DVE max-reduce
      -> exact-equality one-hot (bf16)
  S2b PE-transpose one-hot back to channel-major -> SBUF (f32r)
  S3  CondMul: all 8 experts via TWO fp8 DoubleRow matmuls
      (contraction 256 = [r; h] at 0.5 cyc/col); block-masked w3-table
      matmul folds expert mask + w3 gather; per-pixel dot folded into a
      third fp8 DoubleRow reduce; mask/res rows share one PSUM tile,
      one copy + one DMA per unit.

Precision engineering (relerr ~1.19e-2 vs the 2e-2 gate; x ships f16):
  - weight hi/lo splits give ~22-bit weights where argmax cares (L1 lo
    is f16 scaled by 2^11 against xs to dodge f16 subnormal flush; L2/
    L3 lo in f32r);
  - logits stay f32 through transpose/max/compare so exact-equality
    one-hot ties stay as rare as in fp32;
  - the regression branch (r, cat, w2, w3 gather, final dot) runs in
    fp8e4m3 (x16 prescale, host rescales): its output reaches x_real
    as reg/128, so fp8 noise is invisible at the gate.

Engine balance per unit (~5us slot): Scalar 6 evacuation-activations,
DVE bias/max/eq/mults/row-copy, Pool the fp8 cat cast, PE 17 matmuls
(f16/f32r at 1 cyc/col, fp8 DR at 0.5); PSUM: 2+4+2 banks for the
S1 ring, transpose/CondMul ring, and the packed output rows.

Host side (axon tunnel ~60-80 MB/s, per-array round trips): x ships as
f16, all constants pack into one [128, NCONST] f32 tensor (device-
resident across calls keyed by content hash), outputs pack into one
[BPC, UPB, 1024] f32 tensor ([mask | res] per unit); mask bias+lrelu
and the /(128*16) rescale finish on the host.
"""
import numpy as np

import concourse.bacc as bacc
import concourse.mybir as mybir
import concourse.tile as tile
from concourse.bass_utils import run_bass_kernel_spmd

F32 = mybir.dt.float32
F32R = mybir.dt.float32r
BF16 = mybir.dt.bfloat16
F16 = mybir.dt.float16
F8E4 = mybir.dt.float8e4
AF = mybir.ActivationFunctionType
ALU = mybir.AluOpType
AX = mybir.AxisListType
DR = mybir.MatmulPerfMode.DoubleRow

B, C, W = 16, 128, 8192
NCORES = 8
BPC = B // NCORES          # batches per core
U = 512                    # pixels per pipeline unit
UPB = W // U               # units per batch (16)
H = BPC * UPB              # units per core (32)
CLASSES = 128
EPS = 1e-5
W2SCALE = 16.0             # w2 prescale so fp8e4m3 sees normal range
VSCALE = 16.0              # res-row prescale (fp8 mul path); host divides

_CACHE = {}

# packed-const column layout (single [128, NCONST] f32 input)
CO_W1T = 0
CO_W2T = 128
CO_W3T = 256
CO_R1T = 384
CO_W2P = 512         # 2 groups x [128, 256] DoubleRow weight packs
CO_W3S = 1024        # 2 groups x [128, 128] block-masked w3 tables
CO_V = 1280          # iota + b3
CO_S1 = 1281
CO_B1 = 1282
CO_B2C = 1283
CO_B3C = 1284
CO_SR = 1285
CO_BR = 1286
CO_WLAST = 1287
CO_B2S = 1288        # 2 cols
NCONST = 1290


def _build_nc():
    nc = bacc.Bacc("TRN2", target_bir_lowering=False, debug=False)

    x_d = nc.dram_tensor("x", [BPC, C, W], F16, kind="ExternalInput")
    cpk_d = nc.dram_tensor("cpack", [128, NCONST], F32, kind="ExternalInput")
    # per-unit row pair [mask | res] packed side by side; host applies
    # mask bias+lrelu and the /(128*VSCALE) scale.  f32: no quantization.
    out_d = nc.dram_tensor("out", [BPC, UPB, 2 * U], F32,
                           kind="ExternalOutput")

    with tile.TileContext(nc) as tc:
        with (
            tc.tile_pool(name="consts", bufs=1) as cp,
            tc.tile_pool(name="xin", bufs=1) as xp,
            tc.tile_pool(name="work", bufs=1) as wp,
            tc.tile_pool(name="psA", bufs=2, space="PSUM") as pmA,
            tc.tile_pool(name="psB", bufs=4, space="PSUM") as pmB,
            tc.tile_pool(name="psrow", bufs=1, space="PSUM") as pr,
        ):
            # ---- setup: one DMA for every constant, on-chip casts ----
            cpk = cp.tile([128, NCONST], F32, tag="cpack")
            nc.sync.dma_start(cpk[:], cpk_d[:])

            def csl(col, n=1):
                return cpk[:, col:col + n]

            def f16split(col, name):
                # hi/lo weight split in f32r: the lo residual (~w * 2^-12)
                # underflows f16's subnormal range and would be flushed;
                # f32r's 8-bit exponent keeps it alive.  Moving operands
                # stay f16 (mixed-dtype matmul, still 1 cycle/col).
                wh = cp.tile([128, 128], F32R, tag=f"{name}_h")
                nc.vector.tensor_copy(wh[:], csl(col, 128))
                wl = cp.tile([128, 128], F32R, tag=f"{name}_l")
                nc.vector.tensor_tensor(wl[:], csl(col, 128), wh[:],
                                        ALU.subtract)
                return wh, wl

            s1 = csl(CO_S1)
            b1 = csl(CO_B1)
            b2c = csl(CO_B2C)
            b3c = csl(CO_B3C)
            sr = csl(CO_SR)
            br = csl(CO_BR)
            b2s = [csl(CO_B2S + g) for g in range(2)]

            # setup ordered by first use: w1h/r1 gate the first L1/R1
            # matmuls, so they are cast first; everything else overlaps
            # the early pipeline slots.
            w1h = cp.tile([128, 128], F16, tag="w1_h")
            nc.vector.tensor_copy(w1h[:], csl(CO_W1T, 128))
            r1_16 = cp.tile([128, 128], F16, tag="r1_16")
            nc.vector.tensor_copy(r1_16[:], csl(CO_R1T, 128))
            # L1 lo: f16((w - f16(w)) * 2^11); pairs with xs = x * 2^-11
            # (raw residual would underflow f16 subnormals)
            w1res = cp.tile([128, 128], F32, tag="w1_res")
            nc.vector.tensor_tensor(w1res[:], csl(CO_W1T, 128), w1h[:],
                                    ALU.subtract)
            w1l = cp.tile([128, 128], F16, tag="w1_l")
            nc.vector.tensor_scalar(w1l[:], w1res[:], 2048.0, None, ALU.mult)

            # identity matrices built on-device: iota(j - p) == 0
            iota_i = cp.tile([128, 128], mybir.dt.int32, tag="iota_i")
            nc.gpsimd.iota(iota_i[:], [[1, 128]], base=0,
                           channel_multiplier=-1)
            idn32t = cp.tile([128, 128], F32, tag="idn32")
            nc.gpsimd.tensor_scalar(idn32t[:], iota_i[:], 0, None,
                                    ALU.is_equal)
            idn32 = idn32t[:]

            w2h, w2l = f16split(CO_W2T, "w2")
            w3h, w3l = f16split(CO_W3T, "w3")
            idnbf = cp.tile([128, 128], BF16, tag="idnbf")
            nc.vector.tensor_copy(idnbf[:], idn32)
            wlastr = cp.tile([128, 1], F32R, tag="wlastr")
            nc.vector.tensor_copy(wlastr[:], csl(CO_WLAST))
            v32r = cp.tile([128, 1], F32R, tag="v32r")
            nc.vector.tensor_copy(v32r[:], csl(CO_V))
            # DR reduce weights: [128, 2, 32] with only output row 0 = 1
            ones2_8 = cp.tile([128, 64], F8E4, tag="ones2_8")
            nc.vector.memset(ones2_8[:], 0.0)
            nc.vector.memset(ones2_8[:, 0:1], 1.0)
            nc.vector.memset(ones2_8[:, 32:33], 1.0)
            w2p8 = []
            for g in range(2):
                t = cp.tile([128, 256], F8E4, tag=f"w2p8_{g}")
                nc.vector.tensor_copy(t[:], csl(CO_W2P + g * 256, 256))
                w2p8.append(t)
            w3s = []
            for g in range(2):
                t = cp.tile([128, 128], F32R, tag=f"w3s_{g}")
                nc.vector.tensor_copy(t[:], csl(CO_W3S + g * 128, 128))
                w3s.append(t)

            st = [dict() for _ in range(H)]

            def S0(i):
                """prefetch x + scaled copy for the L1 lo term"""
                b, u = divmod(i, UPB)
                w0 = u * U
                x_t = xp.tile([128, U], F16, tag="x", bufs=5)
                nc.sync.dma_start(x_t[:], x_d[b, :, w0:w0 + U])
                xs_t = xp.tile([128, U], F16, tag="xs", bufs=5)
                if i % 2 == 0:
                    nc.vector.tensor_scalar(xs_t[:], x_t[:], 1.0 / 2048.0,
                                            None, ALU.mult)
                else:
                    nc.scalar.activation(xs_t[:], x_t[:], AF.Identity,
                                         scale=1.0 / 2048.0)
                st[i].update(x=x_t, xs=xs_t, b=b, w0=w0)

            def S1a(i):
                """L1 -> h (f32r), R1 -> r8 (fp8), h8 (Pool)"""
                s = st[i]
                ps = pmA.tile([128, U], F32, tag="mmA")
                nc.tensor.matmul(ps[:], w1h[:], s["x"][:],
                                 start=True, stop=False)
                nc.tensor.matmul(ps[:], w1l[:], s["xs"][:],
                                 start=False, stop=True)
                h_t = wp.tile([128, U], F32R, tag="h", bufs=6)
                nc.scalar.activation(h_t[:], ps[:], AF.Lrelu,
                                     bias=b1, scale=s1, alpha=0.01)

                ps = pmA.tile([128, U], F32, tag="mmA")
                nc.tensor.matmul(ps[:], r1_16[:], s["x"][:],
                                 start=True, stop=True)
                rh8_t = wp.tile([128, 2 * U], F8E4, tag="rh8", bufs=8)
                nc.scalar.activation(rh8_t[:, 0:U], ps[:], AF.Lrelu,
                                     bias=br, scale=sr, alpha=0.01)
                nc.gpsimd.tensor_copy(rh8_t[:, U:2 * U], h_t[:])
                s["h"] = h_t
                s["rh8"] = rh8_t

            def S1b(i):
                """L2 -> x2 (f32r)"""
                s = st[i]
                ps = pmA.tile([128, U], F32, tag="mmA")
                nc.tensor.matmul(ps[:], w2h[:], s["h"][:],
                                 start=True, stop=False)
                nc.tensor.matmul(ps[:], w2l[:], s["h"][:],
                                 start=False, stop=True)
                x2_t = wp.tile([128, U], F32R, tag="x2", bufs=8)
                nc.scalar.activation(x2_t[:], ps[:], AF.Lrelu,
                                     bias=b2c, alpha=0.01)
                s["x2"] = x2_t

            def S1c(i):
                """L3 -> logits l_t (f32, bias via DVE)"""
                s = st[i]
                ps = pmA.tile([128, U], F32, tag="mmA")
                nc.tensor.matmul(ps[:], w3h[:], s["x2"][:],
                                 start=True, stop=False)
                nc.tensor.matmul(ps[:], w3l[:], s["x2"][:],
                                 start=False, stop=True)
                l_t = wp.tile([128, U], F32, tag="lt", bufs=5)
                if i % 4 == 3:
                    nc.scalar.activation(l_t[:], ps[:], AF.Identity, bias=b3c)
                else:
                    nc.vector.tensor_scalar(l_t[:], ps[:], b3c, None, ALU.add)
                s["lt"] = l_t

            def S2a(i):
                """logits -> pixel-major -> per-pixel max -> one-hot"""
                s = st[i]
                ps_lt = pmB.tile([128, U], F32, tag="mmB")
                for j in range(U // 128):
                    nc.tensor.transpose(ps_lt[:, j * 128:(j + 1) * 128],
                                        s["lt"][:, j * 128:(j + 1) * 128],
                                        idn32)
                lt3 = ps_lt[:].rearrange("p (b c) -> p b c", c=128)
                nhb = U // 128
                maxv = wp.tile([128, nhb], F32, tag="maxv", bufs=3)
                nc.vector.tensor_reduce(maxv[:], lt3, AX.X, ALU.max)
                eq_t = wp.tile([128, U], BF16, tag="eq", bufs=4)
                eq3 = eq_t[:].rearrange("p (b c) -> p b c", c=128)
                maxb = maxv[:].unsqueeze(-1).broadcast_to([128, nhb, 128])
                nc.vector.tensor_tensor(eq3, lt3, maxb, ALU.is_equal)
                s["eq"] = eq_t

            def S2b(i):
                """one-hot back to channel-major"""
                s = st[i]
                ps_oh = pmB.tile([128, U], BF16, tag="mmB")
                for j in range(U // 128):
                    nc.tensor.transpose(ps_oh[:, j * 128:(j + 1) * 128],
                                        s["eq"][:, j * 128:(j + 1) * 128],
                                        idnbf[:])
                oh_t = wp.tile([128, U], F32R, tag="oh", bufs=4)
                nc.scalar.copy(oh_t[:], ps_oh[:])
                s["oh"] = oh_t

            def S3(i):
                """CondMul experts (fp8 DoubleRow), w3 select, output rows"""
                s = st[i]
                oh_t, x2_t, rh8_t = s["oh"], s["x2"], s["rh8"]
                rh3 = rh8_t[:].rearrange("p (two n) -> p two n", two=2)
                mul_t = wp.tile([128, 2 * U], F8E4, tag="mul", bufs=2)
                for g in range(2):
                    ps_ly = pmB.tile([128, U], F32, tag="mmB")
                    w3d = w2p8[g][:].rearrange("p (two m) -> p two m", two=2)
                    nc.tensor.matmul(ps_ly[:], w3d, rh3, start=True, stop=True,
                                     perf_mode=DR)
                    ly_g = wp.tile([128, U], F16, tag=f"ly{g}", bufs=2)
                    nc.scalar.activation(ly_g[:], ps_ly[:], AF.Lrelu,
                                         bias=b2s[g], scale=1.0 / W2SCALE,
                                         alpha=0.01)
                    ps_w = pmB.tile([128, U], F32, tag="mmB")
                    nc.tensor.matmul(ps_w[:], w3s[g][:], oh_t[:],
                                     start=True, stop=True)
                    # mul in fp8 x16 (w3s tables are prescaled x16);
                    # host folds the /16 into the final /128 scale
                    nc.vector.tensor_tensor(mul_t[:, g * U:(g + 1) * U],
                                            ly_g[:], ps_w[:], ALU.mult)

                b, u = s["b"], s["w0"] // U
                rows2 = pr.tile([32, 2 * U], F32, tag="rows", name="rows2")
                nc.tensor.matmul(rows2[0:1, 0:U], wlastr[:], x2_t[:],
                                 start=True, stop=True, skip_group_check=True)
                nc.tensor.matmul(rows2[0:1, U:2 * U], v32r[:], oh_t[:],
                                 start=True, stop=False, skip_group_check=True)
                mul3 = mul_t[:].rearrange("p (two n) -> p two n", two=2)
                o3 = ones2_8[:].rearrange("p (two m) -> p two m", two=2)
                nc.tensor.matmul(rows2[0:32, U:2 * U], o3, mul3,
                                 start=False, stop=True, perf_mode=DR,
                                 skip_group_check=True)
                rw = wp.tile([1, 2 * U], F32, tag="rw", bufs=2)
                nc.vector.tensor_copy(rw[:], rows2[0:1, :])
                nc.sync.dma_start(out_d[b, u], rw[:])
                st[i] = {}

            for i in range(H + 7):
                # oldest stages first: their deps resolved slots ago, so
                # each engine queue drains ready work before same-slot
                # dependent work (avoids head-of-line blocking)
                if 0 <= i - 6 < H:
                    S3(i - 6)
                if 0 <= i - 5 < H:
                    S2b(i - 5)
                if 0 <= i - 4 < H:
                    S2a(i - 4)
                if i < H:
                    S0(i)
                if 0 <= i - 1 < H:
                    S1a(i - 1)
                if 0 <= i - 2 < H:
                    S1b(i - 2)
                if 0 <= i - 3 < H:
                    S1c(i - 3)

    nc.compile()
    return nc


def _prep_consts(inputs):
    f32 = np.float32
    cl1_w = np.asarray(inputs['cl1_w'], f32)
    cl1_b = np.asarray(inputs['cl1_b'], f32)
    g1 = np.asarray(inputs['cl1_bn_g'], f32)
    bt1 = np.asarray(inputs['cl1_bn_b'], f32)
    m1 = np.asarray(inputs['cl1_bn_m'], f32)
    v1 = np.asarray(inputs['cl1_bn_v'], f32)
    cl2_w = np.asarray(inputs['cl2_w'], f32)
    cl2_b = np.asarray(inputs['cl2_b'], f32)
    cl3_w = np.asarray(inputs['cl3_w'], f32)
    cl3_b = np.asarray(inputs['cl3_b'], f32)
    reg1_w = np.asarray(inputs['reg1_w'], f32)
    reg1_b = np.asarray(inputs['reg1_b'], f32)
    gr = np.asarray(inputs['reg1_bn_g'], f32)
    btr = np.asarray(inputs['reg1_bn_b'], f32)
    mr = np.asarray(inputs['reg1_bn_m'], f32)
    vr = np.asarray(inputs['reg1_bn_v'], f32)
    w2 = np.asarray(inputs['w2'], f32)      # [8, 256, 32]
    b2 = np.asarray(inputs['b2'], f32)      # [8, 32]
    w3 = np.asarray(inputs['w3'], f32)      # [128, 32, 1]
    b3 = np.asarray(inputs['b3'], f32)      # [128, 1]

    s1 = g1 / np.sqrt(v1 + EPS)
    b1 = (cl1_b - m1) * s1 + bt1
    srv = gr / np.sqrt(vr + EPS)
    brv = (reg1_b - mr) * srv + btr

    cpack = np.zeros((128, NCONST), f32)
    cpack[:, CO_W1T:CO_W1T + 128] = cl1_w.T
    cpack[:, CO_W2T:CO_W2T + 128] = cl2_w.T
    cpack[:, CO_W3T:CO_W3T + 128] = cl3_w[:128].T
    cpack[:, CO_R1T:CO_R1T + 128] = reg1_w.T
    for g in range(2):
        # DoubleRow pack: [:, i*128+m] = w2[4g+s, i*128+p, k], m = 32s+k
        blk = np.zeros((128, 256), f32)
        for s in range(4):
            e = 4 * g + s
            for i in range(2):
                blk[:, i * 128 + s * 32:i * 128 + (s + 1) * 32] = \
                    w2[e, i * 128:(i + 1) * 128, :]
        cpack[:, CO_W2P + g * 256:CO_W2P + (g + 1) * 256] = blk * W2SCALE
        cpack[:, CO_B2S + g][4 * 32:] = 0.0
        bcol = np.zeros(128, f32)
        for s in range(4):
            bcol[s * 32:(s + 1) * 32] = b2[4 * g + s]
        cpack[:, CO_B2S + g] = bcol
        w3sel = np.zeros((128, 128), f32)
        for s in range(4):
            c0 = g * 64 + s * 16
            w3sel[c0:c0 + 16, s * 32:(s + 1) * 32] = w3[c0:c0 + 16, :, 0]
        cpack[:, CO_W3S + g * 128:CO_W3S + (g + 1) * 128] = w3sel * VSCALE
    cpack[:, CO_V] = (np.arange(128, dtype=f32) + b3[:, 0]) * VSCALE
    cpack[:, CO_S1] = s1
    cpack[:, CO_B1] = b1
    cpack[:, CO_B2C] = cl2_b
    cpack[:, CO_B3C] = cl3_b[:128]
    cpack[:, CO_SR] = srv
    cpack[:, CO_BR] = brv
    cpack[:, CO_WLAST] = cl3_w[128]

    return {
        "cpack": cpack,
        "maskb_host": float(cl3_b[128]),
    }


_DISPATCH_CACHE = {}
_DISPATCH_MESH = {}


def _cached_dispatch(nc, n_cores):
    """run_bass_via_pjrt's axon multi-core path with the jitted program
    hoisted out and cached, so repeat kernel() calls skip the per-call
    retrace + XLA re-compile. Execution path / NEFF are identical."""
    key = (id(nc), n_cores)
    d = _DISPATCH_CACHE.get(key)
    if d is not None:
        return d
    import jax
    from jax.experimental.shard_map import shard_map
    from jax.sharding import Mesh, PartitionSpec
    from concourse import bass2jax

    bass2jax.install_neuronx_cc_hook()
    assert nc.dbg_addr is None, "debug build not supported in fast path"
    partition_name = (nc.partition_id_tensor.name
                      if nc.partition_id_tensor else None)
    in_names, out_names, out_avals, zero_specs = [], [], [], []
    for alloc in nc.m.functions[0].allocations:
        if not isinstance(alloc, mybir.MemoryLocationSet):
            continue
        name = alloc.memorylocations[0].name
        if alloc.kind == "ExternalInput":
            if name != partition_name:
                in_names.append(name)
        elif alloc.kind == "ExternalOutput":
            out_names.append(name)
            shape = tuple(alloc.tensor_shape)
            dtype = mybir.dt.np(alloc.dtype)
            out_avals.append(jax.core.ShapedArray(shape, dtype))
            zero_specs.append((shape, dtype))
    n_params = len(in_names)
    n_outs = len(out_avals)
    bind_names = list(in_names) + list(out_names)
    if partition_name is not None:
        bind_names.append(partition_name)
    donate = tuple(range(n_params, n_params + n_outs))

    def _body(*args):
        operands = list(args)
        if partition_name is not None:
            operands.append(bass2jax.partition_id_tensor())
        outs = bass2jax._bass_exec_p.bind(
            *operands,
            out_avals=tuple(out_avals),
            in_names=tuple(bind_names),
            out_names=tuple(out_names),
            lowering_input_output_aliases=(),
            sim_require_finite=True,
            sim_require_nnan=True,
            nc=nc,
        )
        return tuple(outs)

    devices = jax.devices()[:n_cores]
    assert len(devices) == n_cores
    mesh = Mesh(np.asarray(devices), ("core",))
    in_specs = (PartitionSpec("core"),) * (n_params + n_outs)
    out_specs = (PartitionSpec("core"),) * n_outs
    sharded = jax.jit(
        shard_map(_body, mesh=mesh, in_specs=in_specs,
                  out_specs=out_specs, check_rep=False),
        donate_argnums=donate, keep_unused=True,
    )
    d = (sharded, in_names, out_names, out_avals, zero_specs)
    _DISPATCH_CACHE[key] = d
    _DISPATCH_MESH[id(nc)] = mesh
    return d


_STAGE_CACHE = {}


def _stage_resident(name, per_core_arr, n_cores, mesh):
    """Content-hash keyed device residency for static (weight) arrays:
    identical bytes reuse the staged device buffer, changed bytes
    restage (replicating per core). Never applied to activations."""
    import hashlib
    import jax
    from jax.sharding import NamedSharding, PartitionSpec

    h = hashlib.sha1(per_core_arr.tobytes()).digest()
    ent = _STAGE_CACHE.get(name)
    if ent is not None and ent[0] == h:
        return ent[1]
    garr = np.tile(per_core_arr, (n_cores,) + (1,) * (per_core_arr.ndim - 1))
    buf = jax.device_put(garr, NamedSharding(mesh, PartitionSpec("core")))
    buf.block_until_ready()
    _STAGE_CACHE[name] = (h, buf)
    return buf


def _run_fast(nc, global_ins, n_cores, resident=("cpack",),
              prestaged_zeros=None):
    """global_ins: name -> global (n_cores*d0, ...) array, except
    names in `resident`, which are per-core and replicated on miss."""
    sharded, in_names, out_names, out_avals, zero_specs = _cached_dispatch(
        nc, n_cores)
    args = []
    for name in in_names:
        arr = global_ins[name]
        if name in resident:
            arr = _stage_resident(name, arr, n_cores, _DISPATCH_MESH[id(nc)])
        args.append(arr)
    concat_zeros = prestaged_zeros
    if concat_zeros is None:
        concat_zeros = [
            np.zeros((n_cores * shape[0], *shape[1:]), dtype)
            for shape, dtype in zero_specs
        ]
    out_arrs = sharded(*args, *concat_zeros)
    return {
        name: np.asarray(out_arrs[i]).reshape(n_cores, *out_avals[i].shape)
        for i, name in enumerate(out_names)
    }


def _run(inputs, trace=False, **kw):
    key = "nc_v2"
    if key not in _CACHE:
        _CACHE[key] = _build_nc()
    nc = _CACHE[key]

    consts = _prep_consts(inputs)
    maskb = consts.pop("maskb_host")
    cpack = consts["cpack"]
    x_in = np.asarray(inputs['x_in'], np.float32).reshape(B, C, W)

    if trace or kw:
        x_ship = x_in.astype(np.float16)
        in_maps = []
        for c in range(NCORES):
            m = {"cpack": cpack, "x": x_ship[c * BPC:(c + 1) * BPC]}
            in_maps.append(m)
        res = run_bass_kernel_spmd(nc, in_maps, list(range(NCORES)),
                                   trace=trace, **kw)
        out = np.stack([res.results[c]["out"] for c in range(NCORES)])
    else:
        # pipelined staging: enqueue the (donated, fresh-per-call)
        # zero output buffers first so their transfer hides under
        # the f16 encode, then encode each core's x slice and
        # enqueue its (async) transfer while encoding the next
        import jax
        from jax.sharding import NamedSharding, PartitionSpec
        _, _, _, _, zero_specs = _cached_dispatch(nc, NCORES)
        mesh = _DISPATCH_MESH[id(nc)]
        shd = NamedSharding(mesh, PartitionSpec("core"))
        devices = list(mesh.devices.flatten())
        zbufs = [
            jax.device_put(
                np.zeros((NCORES * shape[0], *shape[1:]), dtype), shd)
            for shape, dtype in zero_specs
        ]
        shards = [
            jax.device_put(
                x_in[c * BPC:(c + 1) * BPC].astype(np.float16),
                devices[c])
            for c in range(NCORES)
        ]
        x_ship = jax.make_array_from_single_device_arrays(
            (B, C, W), shd, shards)
        global_ins = {"cpack": cpack, "x": x_ship}
        outs = _run_fast(nc, global_ins, NCORES, prestaged_zeros=zbufs)
        out = outs["out"]             # [NCORES, 2, BPC, W]
        res = type("R", (), {"exec_time_ns": None, "mean_exec_time_ns": None,
                             "max_exec_time_core_id": None,
                             "results": outs})()

    # out dims [NCORES, BPC, UPB, 2U]: cols 0:U mask, U:2U res
    mask = out[..., 0:U].reshape(B, W).astype(np.float32)
    xr = out[..., U:2 * U].reshape(B, W).astype(np.float32)
    # host-side finishing
    mask = mask + maskb
    mask = np.where(mask >= 0, mask, 0.01 * mask)
    xr = xr * (1.0 / (CLASSES * VSCALE))
    out_xr = xr.reshape(B, 1, 1, W)
    out_mask = mask.reshape(B, 1, 1, W)
    return (out_xr, out_mask), res


def kernel(**inputs):
    (out_xr, out_mask), _ = _run(inputs)
    return (out_xr, out_mask)
